# revision 27
# baseline (speedup 1.0000x reference)
"""Trainium2 Bass kernel for nn_AttShare: dual-stream 1x1-conv attention.

Full-input contract: kernel(**inputs) takes the complete tensors from
setup_inputs() and returns (out1, out2) exactly like the reference.

Sharding (8 cores): 4 independent (batch, stream) attention units x 2-way
query-row split.  Each core gets the full x=[256,4096] of its unit, HOST-
ROTATED so its 2048 query columns come first; it produces
out = gamma * (V @ softmax(Q K^T)^T)[:, 0:2048] + x[:, 0:2048].
(Attention contracts over all keys, so the key/value column order is
irrelevant; the host scatters the output back to the right columns.)

Key simplification: the reference adds a per-row bias (q . g) to the logits
before a row-softmax.  softmax is shift-invariant per row, so the entire
global-gating branch (pooled means -> MLP -> sigmoid -> bias) cancels and is
not computed.  The k-projection bias also shifts logits uniformly per row
and cancels; the q bias does not and is applied.  The v bias adds
gamma*vb[c] (softmax rows sum to 1); it is folded into the V^T tiles;
gamma itself is folded into the finalize scalar_tensor_tensor (exact).

Precision: everything rides bf16 except the PSUM accumulations (f32) and
the residual, which is reconstructed exactly from a bf16 hi+lo split of x
(xb + xlo == fp32 x to ~2^-17).  The q projection also uses the hi+lo
split (two bf16 matmul passes == fp32r's 2 cycles/col, but no fp32 DMA);
k/v project from xb alone.  Measured accuracy ~1.25e-2 relative
(tolerance 2e-2); the dominant term is the bf16 rounding of q/k.

On-core dataflow (per core):
  proj:  kk = Wk_dup @ xb (+kb)            [128, 4096] -> bf16 (k dup'd on
         qq = Wq_dup @ (xb+xlo) (+qb)      [128, 2048] -> bf16  both halves
         vt = xb^T Wv^T (+vb via Vector)   [128 j, 32, 256] bf16 for packing)
  attn (4 phases of 512 query columns, keys streamed in row-packed pairs
        of 128-key chunks, software-pipelined in double-pair steps):
         S^T = kk_j^T @ qq  (K=64, PE rows 0-63 / 64-127 run CONCURRENTLY
               via row tiling; both pairs' QK groups issued back-to-back
               to halve the QK<->PV weight-load transitions)
         E = exp(S^T)  (one [128,1024] ScalarE pass per pair covering both
               halves; PSUM -> bf16 SBUF; no max-shift needed: |S|<~60 and
               the denominator normalizes later)
         z2 += E  (Vector, one [128,1024] bf16 add per pair, incl. pair 15)
         out_psum[c,i] += vt_j^T @ E  (bf16 matmuls, PSUM-resident)
  finalize per phase: Z colsum+broadcast via 2 all-ones bf16 lhsT matmuls
  (emitted between the last pair's PV halves), fast reciprocal (Vector,
  18-bit), out = (po * gamma) * recip + xb + xlo, DMA out split across
  both rings by channel half.  Middle phases put the +x adds on Pool; the
  last phase adds a Pool-precomputed r_last = xb+xlo on Vector (short
  tail).  PSUM budget 8 banks: 2x2 phase-rotated output accumulators +
  2x2-bank S^T slots, so a phase's finalize never blocks the next phase's
  PV.

Head: all tensors are pre-permuted on host to partition-major [128, 2, n]
bf16 so every DMA is a dense descriptor; the two cin-halves ride the SP
and Activation hardware DMA rings (~150GB/s each); xlo chunks 1-3 trail
the queue (their qq slices are consumed last).
"""

import os
import sys

import numpy as np

for _p in ("/opt/trn_rl_repo", os.path.expanduser("~/.axon_site/_ro/trn_rl_repo")):
    if os.path.isdir(_p) and _p not in sys.path:
        sys.path.insert(0, _p)

import concourse.bass as bass  # noqa: E402
import concourse.bacc as bacc  # noqa: E402
import concourse.mybir as mybir  # noqa: E402
import concourse.tile as tile  # noqa: E402

P = 128
C = 256         # channels
CQ = 64         # q/k channels
N = 4096        # H*W
NI = 2048       # query rows per core
PH = 512        # query columns processed per phase
B, H, W = 2, 64, 64
F32 = mybir.dt.float32
BF16 = mybir.dt.bfloat16


def _emit(tc, aps):
    nc = tc.nc
    import contextlib

    (xb_d, xlo_d, wq_d, wk_d, wv_d, qb_d, kb_d, vbb_d, gbc_d, out_d) = aps
    EXP = mybir.ActivationFunctionType.Exp
    IDENT = mybir.ActivationFunctionType.Identity
    MUL = mybir.AluOpType.mult
    ADD = mybir.AluOpType.add

    with contextlib.ExitStack() as ctx:
        singles = ctx.enter_context(tc.tile_pool(name="singles", bufs=1))
        pp = ctx.enter_context(tc.tile_pool(name="pp", bufs=2, space="PSUM"))
        p_s = ctx.enter_context(tc.tile_pool(name="p_s", bufs=3, space="PSUM"))
        etp = ctx.enter_context(tc.tile_pool(name="etp", bufs=8))
        zp = ctx.enter_context(tc.tile_pool(name="zp", bufs=4))
        outp = ctx.enter_context(tc.tile_pool(name="outp", bufs=4))

        # ---- loads --------------------------------------------------------------
        xb_sb = singles.tile([P, 2, N], BF16)     # full rotated x, bf16 hi
        xlo_sb = singles.tile([P, 2, NI], BF16)   # query-half lo correction
        wq_sb = singles.tile([P, 2, P], BF16)
        wk_sb = singles.tile([P, 2, P], BF16)
        wv_sb = singles.tile([P, 2, C], BF16)

        kb_sb = singles.tile([P, 1], F32)
        qb_sb = singles.tile([P, 1], F32)
        gamma_bc = singles.tile([P, 1], F32)   # host pre-broadcast
        vb_bc = singles.tile([P, C], F32)      # host pre-broadcast

        def ld(queue, sb, dram, o, c):
            queue.dma_start(out=sb[:, o:o + 1, bass.ts(c, 512)],
                            in_=dram[:][:, o:o + 1, bass.ts(c, 512)])

        def ld2(queue, sb, dram, c):
            queue.dma_start(out=sb[:, :, bass.ts(c, 512)],
                            in_=dram[:][:, :, bass.ts(c, 512)])

        # Three-queue head: SP carries o=0 x halves (issue-only engine,
        # front-loaded); Activation carries o=1 halves, paced inside the
        # projection emission so descriptor writes never starve the
        # identity activations; Pool carries the tiny params, both
        # pre-broadcasts, xlo, and the last two x chunks.  ~1.1MB/queue.
        nc.gpsimd.dma_start(out=qb_sb, in_=qb_d[:])
        nc.gpsimd.dma_start(out=kb_sb, in_=kb_d[:])
        nc.gpsimd.dma_start(out=gamma_bc, in_=gbc_d[:])
        nc.gpsimd.dma_start(out=vb_bc, in_=vbb_d[:])
        nc.gpsimd.dma_start(out=xlo_sb[:, :, bass.ds(0, 512)],
                            in_=xlo_d[:][:, :, bass.ds(0, 512)])
        ld2(nc.gpsimd, xb_sb, xb_d, 6)
        ld2(nc.gpsimd, xb_sb, xb_d, 7)
        nc.gpsimd.dma_start(out=xlo_sb[:, :, bass.ds(512, 1536)],
                            in_=xlo_d[:][:, :, bass.ds(512, 1536)])

        nc.sync.dma_start(out=wk_sb, in_=wk_d[:])
        nc.sync.dma_start(out=wq_sb, in_=wq_d[:])
        for c in range(6):
            ld(nc.sync, xb_sb, xb_d, 0, c)
        nc.scalar.dma_start(out=wv_sb, in_=wv_d[:])
        ld(nc.scalar, xb_sb, xb_d, 1, 0)
        ld(nc.scalar, xb_sb, xb_d, 1, 1)

        ones_b = singles.tile([P, P], BF16)   # all-ones bf16 lhsT: Z colsum
        nc.vector.memset(ones_b, 1.0)

        # ---- projections --------------------------------------------------------
        # qq/kk stored bf16: the QK matmuls then stream 1 cycle/col with
        # single-pass weight loads.  All projection matmuls are bf16 (fp32r
        # runs 2 cycles/col on HW); q keeps full x precision via the hi+lo
        # split (two accumulating bf16 passes).
        qq_sb = singles.tile([P, NI], BF16)    # [q; q] duplicated across halves
        kk_sb = singles.tile([P, N], BF16)     # [k; k] duplicated across halves
        vt_sb = singles.tile([P, N // P, C], BF16)   # V^T: [j, c], +vb folded

        def qq_slice(s):
            ps = pp.tile([P, 512], F32, tag="pp", name=f"qq_ps_{s}")
            nc.tensor.matmul(ps, wq_sb[:, 0], xb_sb[:, 0, bass.ts(s, 512)],
                             start=True, stop=False)
            nc.tensor.matmul(ps, wq_sb[:, 0], xlo_sb[:, 0, bass.ts(s, 512)],
                             start=False, stop=False)
            nc.tensor.matmul(ps, wq_sb[:, 1], xb_sb[:, 1, bass.ts(s, 512)],
                             start=False, stop=False)
            nc.tensor.matmul(ps, wq_sb[:, 1], xlo_sb[:, 1, bass.ts(s, 512)],
                             start=False, stop=True)
            nc.scalar.activation(out=qq_sb[:, bass.ts(s, 512)], in_=ps,
                                 func=IDENT, bias=qb_sb, scale=1.0)

        def kk_slice(s):
            ps = pp.tile([P, 512], F32, tag="pp", name=f"kk_ps_{s}")
            nc.tensor.matmul(ps, wk_sb[:, 0], xb_sb[:, 0, bass.ts(s, 512)],
                             start=True, stop=False)
            nc.tensor.matmul(ps, wk_sb[:, 1], xb_sb[:, 1, bass.ts(s, 512)],
                             start=False, stop=True)
            nc.scalar.activation(out=kk_sb[:, bass.ts(s, 512)], in_=ps,
                                 func=IDENT, bias=kb_sb, scale=1.0)

        def vt_chunk(j):
            ps = pp.tile([P, C], F32, tag="pp", name=f"vt_ps_{j}")
            nc.tensor.matmul(ps, xb_sb[:, 0, bass.ts(j, P)], wv_sb[:, 0],
                             start=True, stop=False)
            nc.tensor.matmul(ps, xb_sb[:, 1, bass.ts(j, P)], wv_sb[:, 1],
                             start=False, stop=True)
            nc.vector.tensor_add(vt_sb[:, j], ps, vb_bc)

        # consume xb strictly in chunk-arrival order; the o=1 half of
        # chunk s+2 is issued from the scalar queue as chunk s is consumed
        # (chunks 6-7 ride the Pool queue instead); qq slices 1-3 are
        # emitted mid-phase, right before the pipeline needs them
        kk_slice(0)
        for j in range(4):
            vt_chunk(j)
        qq_slice(0)
        for s in range(1, N // 512):
            if s + 1 < 6:
                ld(nc.scalar, xb_sb, xb_d, 1, s + 1)
            kk_slice(s)
            for j in range(4 * s, 4 * s + 4):
                vt_chunk(j)
        qq_slice(1)
        qq_slice(2)
        qq_slice(3)

        # full residual r = xb + xlo (exact fp32 x), precomputed on Pool
        # (idle during projections) so each phase's finalize needs only one
        # add per channel half; chunked to follow the xlo DMA arrivals
        r_sb = singles.tile([P, 2, NI], F32)
        for c in range(4):
            nc.gpsimd.tensor_add(r_sb[:, :, bass.ts(c, 512)],
                                 xb_sb[:, :, bass.ts(c, 512)],
                                 xlo_sb[:, :, bass.ts(c, 512)])

        # ---- attention ----------------------------------------------------------
        # Row-packed QK: pair (jA, jB) = (2t, 2t+1); jA on PE rows 0-63, jB on
        # rows 64-127 (via the duplicated q/k halves), running concurrently.
        NPAIR = N // P // 2   # 16 pairs per phase
        NPH = NI // PH        # 4 phases

        def issue_pair(ph, t):
            # One PSUM tile holds both halves' S^T slices ([P, 2, 512]); the
            # two K=64 QK matmuls row-pack (rows 0-63 / 64-127) and a SINGLE
            # [128, 1024] exp covers both halves (amortizes ScalarE's fixed
            # per-instruction overhead -- ScalarE is the co-critical engine).
            i0 = ph * PH
            ps = p_s.tile([P, 2, PH], F32, tag="s", name=f"ps_{ph}_{t}")
            for h, j in ((0, 2 * t), (1, 2 * t + 1)):
                lo = h * CQ
                nc.tensor.matmul(
                    ps[:, h],
                    kk_sb[lo:lo + CQ, bass.ts(j, P)],
                    qq_sb[lo:lo + CQ, bass.ds(i0, PH)],
                    start=True, stop=True)
            et = etp.tile([P, 2, PH], BF16, tag="et", name=f"et_{ph}_{t}")
            nc.scalar.activation(out=et, in_=ps, func=EXP, scale=1.0)
            return et

        def pv_half(po, t, h, et):
            j = 2 * t + h
            for cc in range(C // P):
                nc.tensor.matmul(
                    po[cc],
                    vt_sb[:, j, bass.ts(cc, P)],
                    et[:, h],
                    start=(t == 0 and h == 0), stop=(t == NPAIR - 1 and h == 1))

        def finalize_z(ph, z2, et15=None):
            # Z colsum + partition-broadcast via all-ones bf16 lhsT matmuls
            # accumulated into one PSUM tile; emitted between the last
            # pair's two PV halves so the reciprocal overlaps the remaining
            # PV stream.  For the final phase the last pair's exp tile is
            # summed directly by the PE (et15) so the colsum can issue a
            # full pair earlier, pulling the reciprocal off the tail.
            pzb = p_s.tile([P, PH], F32, tag="s", name=f"pzb_{ph}")
            nc.tensor.matmul(pzb, ones_b, z2[:, 0], start=True, stop=False)
            nc.tensor.matmul(pzb, ones_b, z2[:, 1],
                             start=False, stop=et15 is None)
            if et15 is not None:
                nc.tensor.matmul(pzb, ones_b, et15[:, 0],
                                 start=False, stop=False)
                nc.tensor.matmul(pzb, ones_b, et15[:, 1],
                                 start=False, stop=True)
            zbc = zp.tile([P, PH], F32, tag="zbc", name=f"zbc_{ph}")
            nc.vector.reciprocal_approx_fast(out=zbc, in_=pzb)
            return zbc

        def finalize_out(ph, po, zbc):
            # ob = (po * gamma) * (1/Z) via one STT per channel half, then a
            # single +r residual add.  Middle phases put the adds on Pool
            # while the PE begins the next phase (po PSUM banks rotate
            # between phases); the last phase keeps them on Vector (shorter
            # tail).  DMA issues ride the sync queue and the engine that
            # produced the data -- NEVER the scalar queue, whose in-order
            # stream would stall the critical exps behind the DMA's wait.
            last = ph == NPH - 1
            sl_i = bass.ds(ph * PH, PH)
            ob = outp.tile([P, 2, PH], F32, tag="ob", name=f"ob_{ph}")
            for cc in range(C // P):
                nc.vector.scalar_tensor_tensor(
                    out=ob[:, cc], in0=po[cc], scalar=gamma_bc, in1=zbc,
                    op0=MUL, op1=MUL)
                if last:
                    nc.vector.tensor_add(ob[:, cc], ob[:, cc],
                                         r_sb[:, cc, sl_i])
                else:
                    nc.gpsimd.tensor_add(ob[:, cc], ob[:, cc],
                                         r_sb[:, cc, sl_i])
                q = nc.sync if cc == 0 else nc.gpsimd
                q.dma_start(out=out_d[:][:, cc:cc + 1, sl_i],
                            in_=ob[:, cc:cc + 1])

        # Software pipeline over double-pair steps: per step, issue QK/exp
        # for the NEXT two pairs back-to-back, then run all 8 PV matmuls of
        # the current two pairs.  Batching the QK groups halves the number of
        # QK<->PV transitions (each costs ~300-400ns of weight-load/drain
        # exposure), and the ~2.2us of exp still fits inside the step.
        all_pairs = [(ph, t) for ph in range(NPH) for t in range(NPAIR)]
        pend = {pr: issue_pair(*pr) for pr in all_pairs[:2]}
        z2_by_ph = {}
        po_by_ph = {}
        for i in range(0, len(all_pairs), 2):
            pra, prb = all_pairs[i], all_pairs[i + 1]
            ph = pra[0]
            if pra[1] == 0:
                z2_by_ph[ph] = zp.tile([P, 2, PH], BF16, tag="z2",
                                       name=f"z2_{ph}")
                po_by_ph[ph] = [pp.tile([P, PH], F32, tag="pp",
                                        name=f"po_{ph}_{cc}")
                                for cc in range(C // P)]
            z2 = z2_by_ph[ph]
            po = po_by_ph[ph]
            eta, etb = pend.pop(pra), pend.pop(prb)
            for pr in all_pairs[i + 2:i + 4]:
                pend[pr] = issue_pair(*pr)
            for t, et in ((pra[1], eta), (prb[1], etb)):
                if t == 0:
                    nc.vector.tensor_copy(z2, et)
                elif t != NPAIR - 1:
                    nc.vector.tensor_add(z2, z2, et)
                if t == NPAIR - 1:
                    # 4-term colsum before pair 15's PVs: the reciprocal and
                    # the finalize STTs overlap the PV stream, so the next
                    # phase's first PV never waits the po bank rotation
                    zbc = finalize_z(ph, z2, et15=et)
                    pv_half(po, t, 0, et)
                    pv_half(po, t, 1, et)
                    finalize_out(ph, po, zbc)
                else:
                    pv_half(po, t, 0, et)
                    pv_half(po, t, 1, et)


def _build_nc():
    nc = bacc.Bacc(trn_type="TRN2", target_bir_lowering=False, debug=False)
    aps = (
        nc.declare_dram_parameter("xb", [P, 2, N], BF16, isOutput=False),
        nc.declare_dram_parameter("xlo", [P, 2, NI], BF16, isOutput=False),
        nc.declare_dram_parameter("wq", [P, 2, P], BF16, isOutput=False),
        nc.declare_dram_parameter("wk", [P, 2, P], BF16, isOutput=False),
        nc.declare_dram_parameter("wv", [P, 2, C], BF16, isOutput=False),
        nc.declare_dram_parameter("qb", [P, 1], F32, isOutput=False),
        nc.declare_dram_parameter("kb", [P, 1], F32, isOutput=False),
        nc.declare_dram_parameter("vb_bc", [P, C], F32, isOutput=False),
        nc.declare_dram_parameter("gamma_bc", [P, 1], F32, isOutput=False),
        nc.declare_dram_parameter("out", [P, 2, NI], F32, isOutput=True),
    )
    with tile.TileContext(nc) as tc:
        _emit(tc, aps)
    nc.compile()
    return nc


_NC_CACHE = {}


def get_nc():
    if "nc" not in _NC_CACHE:
        _NC_CACHE["nc"] = _build_nc()
    return _NC_CACHE["nc"]


def _pmajor(a, free):
    """[256, free] -> contiguous [128, 2, free] partition-major view."""
    return np.ascontiguousarray(a.reshape(2, P, free).transpose(1, 0, 2))


def make_in_maps(inputs):
    """Build the 8 per-core input maps from the full problem inputs."""
    import ml_dtypes
    f = np.float32
    bf = ml_dtypes.bfloat16
    x_streams = [
        np.ascontiguousarray(inputs["input1"].reshape(B, C, N), dtype=f),
        np.ascontiguousarray(inputs["input2"].reshape(B, C, N), dtype=f),
    ]
    wsets = []
    for s in ("1", "2"):
        qw = np.asarray(inputs[f"q{s}_w"], dtype=f)
        kw = np.asarray(inputs[f"k{s}_w"], dtype=f)
        vw = np.asarray(inputs[f"v{s}_w"], dtype=f)
        qb = np.asarray(inputs[f"q{s}_b"], dtype=f)
        kb = np.asarray(inputs[f"k{s}_b"], dtype=f)
        vb = np.asarray(inputs[f"v{s}_b"], dtype=f)
        wsets.append(dict(
            wq=_pmajor(np.concatenate([qw, qw], 0).T.astype(bf), P),
            wk=_pmajor(np.concatenate([kw, kw], 0).T.astype(bf), P),
            wv=_pmajor(vw.T.astype(bf), C),
            qb=np.ascontiguousarray(np.concatenate([qb, qb])[:, None]),
            kb=np.ascontiguousarray(np.concatenate([kb, kb])[:, None]),
            vb_bc=np.ascontiguousarray(np.broadcast_to(vb[None, :], (P, C)),
                                       dtype=f),
        ))
    gamma = np.asarray(inputs["gamma"], dtype=f).reshape(1, 1)
    gamma_bc = np.ascontiguousarray(np.broadcast_to(gamma, (P, 1)))

    in_maps = []
    for core in range(8):
        u, h = core // 2, core % 2
        b, s = u // 2, u % 2
        xs = x_streams[s][b]
        # rotate so this core's 2048 query columns come first (attention
        # contracts over all keys, so key/value column order is irrelevant)
        xrot = np.concatenate([xs[:, h * NI:], xs[:, :h * NI]], axis=1)
        xb = xrot.astype(bf)
        xlo = (xrot[:, :NI] - xb[:, :NI].astype(f)).astype(bf)
        m = dict(wsets[s])
        m["xb"] = _pmajor(xb, N)
        m["xlo"] = _pmajor(xlo, NI)
        m["gamma_bc"] = gamma_bc
        in_maps.append(m)
    return in_maps


def assemble(results, inputs):
    """Stitch the 8 per-core [128, 2, 2048] outputs into (out1, out2)."""
    outs = [np.empty((B, C, N), np.float32) for _ in range(2)]
    for core in range(8):
        u, h = core // 2, core % 2
        b, s = u // 2, u % 2
        o = results[core]["out"].transpose(1, 0, 2).reshape(C, NI)
        outs[s][b][:, h * NI:(h + 1) * NI] = o
    out1 = outs[0].reshape(B, C, H, W)
    out2 = outs[1].reshape(B, C, H, W)
    return out1, out2


def kernel(**inputs):
    from concourse.bass_utils import run_bass_kernel_spmd

    nc = get_nc()
    in_maps = make_in_maps(inputs)
    res = run_bass_kernel_spmd(nc, in_maps, list(range(8)))
    return assemble(res.results, inputs)


# revision 28
# speedup vs baseline: 1.0911x; 1.0911x over previous
"""Trainium2 Bass kernel for nn_AttShare: dual-stream 1x1-conv attention.

Full-input contract: kernel(**inputs) takes the complete tensors from
setup_inputs() and returns (out1, out2) exactly like the reference.

Sharding (8 cores): 4 independent (batch, stream) attention units x 2-way
query-row split.  Each core gets the full x=[256,4096] of its unit, HOST-
ROTATED so its 2048 query columns come first; it produces
out = gamma * (V @ softmax(Q K^T)^T)[:, 0:2048] + x[:, 0:2048].
(Attention contracts over all keys, so the key/value column order is
irrelevant; the host scatters the output back to the right columns.)

Key simplification: the reference adds a per-row bias (q . g) to the logits
before a row-softmax.  softmax is shift-invariant per row, so the entire
global-gating branch (pooled means -> MLP -> sigmoid -> bias) cancels and is
not computed.  The k-projection bias also shifts logits uniformly per row
and cancels; the q bias does not and is applied.  The v bias adds
gamma*vb[c] (softmax rows sum to 1); it is folded into the V^T tiles;
gamma itself is folded into the finalize scalar_tensor_tensor (exact).

Precision: everything rides bf16 except the PSUM accumulations (f32) and
the residual, which is reconstructed exactly from a bf16 hi+lo split of x
(xb + xlo == fp32 x to ~2^-17).  The q projection also uses the hi+lo
split (two bf16 matmul passes == fp32r's 2 cycles/col, but no fp32 DMA);
k/v project from xb alone.  Measured accuracy ~1.25e-2 relative
(tolerance 2e-2); the dominant term is the bf16 rounding of q/k.

On-core dataflow (per core):
  proj:  kk = Wk_dup @ xb (+kb)            [128, 4096] -> bf16 (k dup'd on
         qq = Wq_dup @ (xb+xlo) (+qb)      [128, 2048] -> bf16  both halves
         vt = xb^T Wv^T (+vb via Vector)   [128 j, 32, 256] bf16 for packing)
  attn (4 phases of 512 query columns, keys streamed in row-packed pairs
        of 128-key chunks, software-pipelined in double-pair steps):
         S^T = kk_j^T @ qq  (K=64, PE rows 0-63 / 64-127 run CONCURRENTLY
               via row tiling; both pairs' QK groups issued back-to-back
               to halve the QK<->PV weight-load transitions)
         E = exp(S^T)  (one [128,1024] ScalarE pass per pair covering both
               halves; PSUM -> bf16 SBUF; no max-shift needed: |S|<~60 and
               the denominator normalizes later)
         z2 += E  (Vector, one [128,1024] bf16 add per pair, incl. pair 15)
         out_psum[c,i] += vt_j^T @ E  (bf16 matmuls, PSUM-resident)
  finalize per phase: Z colsum+broadcast via 2 all-ones bf16 lhsT matmuls
  (emitted between the last pair's PV halves), fast reciprocal (Vector,
  18-bit), out = (po * gamma) * recip + xb + xlo, DMA out split across
  both rings by channel half.  Middle phases put the +x adds on Pool; the
  last phase adds a Pool-precomputed r_last = xb+xlo on Vector (short
  tail).  PSUM budget 8 banks: 2x2 phase-rotated output accumulators +
  2x2-bank S^T slots, so a phase's finalize never blocks the next phase's
  PV.

Head: all tensors are pre-permuted on host to partition-major [128, 2, n]
bf16 so every DMA is a dense descriptor; the two cin-halves ride the SP
and Activation hardware DMA rings (~150GB/s each); xlo chunks 1-3 trail
the queue (their qq slices are consumed last).
"""

import os
import sys

import numpy as np

for _p in ("/opt/trn_rl_repo", os.path.expanduser("~/.axon_site/_ro/trn_rl_repo")):
    if os.path.isdir(_p) and _p not in sys.path:
        sys.path.insert(0, _p)

import concourse.bass as bass  # noqa: E402
import concourse.bacc as bacc  # noqa: E402
import concourse.mybir as mybir  # noqa: E402
import concourse.tile as tile  # noqa: E402

P = 128
C = 256         # channels
CQ = 64         # q/k channels
N = 4096        # H*W
NI = 2048       # query rows per core
PH = 512        # query columns processed per phase
B, H, W = 2, 64, 64
F32 = mybir.dt.float32
BF16 = mybir.dt.bfloat16


def _emit(tc, aps):
    nc = tc.nc
    import contextlib

    (xb_d, xlo_d, wq_d, wk_d, wv_d, qb_d, kb_d, vbb_d, gbc_d, out_d) = aps
    EXP = mybir.ActivationFunctionType.Exp
    IDENT = mybir.ActivationFunctionType.Identity
    MUL = mybir.AluOpType.mult
    ADD = mybir.AluOpType.add

    with contextlib.ExitStack() as ctx:
        singles = ctx.enter_context(tc.tile_pool(name="singles", bufs=1))
        pp = ctx.enter_context(tc.tile_pool(name="pp", bufs=4, space="PSUM"))
        p_s = ctx.enter_context(tc.tile_pool(name="p_s", bufs=2, space="PSUM"))
        etp = ctx.enter_context(tc.tile_pool(name="etp", bufs=8))
        zp = ctx.enter_context(tc.tile_pool(name="zp", bufs=4))
        outp = ctx.enter_context(tc.tile_pool(name="outp", bufs=4))

        # ---- loads --------------------------------------------------------------
        xb_sb = singles.tile([P, 2, N], BF16)     # full rotated x, bf16 hi
        xlo_sb = singles.tile([P, 2, NI], BF16)   # query-half lo correction
        wq_sb = singles.tile([P, 2, P], BF16)
        wk_sb = singles.tile([P, 2, P], BF16)
        wv_sb = singles.tile([P, 2, C], BF16)

        kb_sb = singles.tile([P, 1], F32)
        qb_sb = singles.tile([P, 1], F32)
        gamma_bc = singles.tile([P, 1], F32)   # host pre-broadcast
        vb_bc = singles.tile([P, C], F32)      # host pre-broadcast

        def ld(queue, sb, dram, o, c):
            queue.dma_start(out=sb[:, o:o + 1, bass.ts(c, 512)],
                            in_=dram[:][:, o:o + 1, bass.ts(c, 512)])

        def ld2(queue, sb, dram, c):
            queue.dma_start(out=sb[:, :, bass.ts(c, 512)],
                            in_=dram[:][:, :, bass.ts(c, 512)])

        # Three-queue head: SP carries o=0 x halves (issue-only engine,
        # front-loaded); Activation carries o=1 halves, paced inside the
        # projection emission so descriptor writes never starve the
        # identity activations; Pool carries the tiny params, both
        # pre-broadcasts, xlo, and the last two x chunks.  ~1.1MB/queue.
        nc.gpsimd.dma_start(out=qb_sb, in_=qb_d[:])
        nc.gpsimd.dma_start(out=kb_sb, in_=kb_d[:])
        nc.gpsimd.dma_start(out=gamma_bc, in_=gbc_d[:])
        nc.gpsimd.dma_start(out=vb_bc, in_=vbb_d[:])
        nc.gpsimd.dma_start(out=xlo_sb[:, :, bass.ds(0, 512)],
                            in_=xlo_d[:][:, :, bass.ds(0, 512)])
        ld2(nc.gpsimd, xb_sb, xb_d, 6)
        ld2(nc.gpsimd, xb_sb, xb_d, 7)
        nc.gpsimd.dma_start(out=xlo_sb[:, :, bass.ds(512, 1536)],
                            in_=xlo_d[:][:, :, bass.ds(512, 1536)])

        nc.sync.dma_start(out=wk_sb, in_=wk_d[:])
        nc.sync.dma_start(out=wq_sb, in_=wq_d[:])
        for c in range(6):
            ld(nc.sync, xb_sb, xb_d, 0, c)
        nc.scalar.dma_start(out=wv_sb, in_=wv_d[:])
        ld(nc.scalar, xb_sb, xb_d, 1, 0)
        ld(nc.scalar, xb_sb, xb_d, 1, 1)

        ones_b = singles.tile([P, P], BF16)   # all-ones bf16 lhsT: Z colsum
        nc.vector.memset(ones_b, 1.0)

        # ---- projections --------------------------------------------------------
        # qq/kk stored bf16: the QK matmuls then stream 1 cycle/col with
        # single-pass weight loads.  All projection matmuls are bf16 (fp32r
        # runs 2 cycles/col on HW); q keeps full x precision via the hi+lo
        # split (two accumulating bf16 passes).
        qq_sb = singles.tile([P, NI], BF16)    # [q; q] duplicated across halves
        kk_sb = singles.tile([P, N], BF16)     # [k; k] duplicated across halves
        vt_sb = singles.tile([P, N // P, C], BF16)   # V^T: [j, c], +vb folded

        def qq_slice(s):
            ps = pp.tile([P, 512], F32, tag="pp", name=f"qq_ps_{s}")
            nc.tensor.matmul(ps, wq_sb[:, 0], xb_sb[:, 0, bass.ts(s, 512)],
                             start=True, stop=False)
            nc.tensor.matmul(ps, wq_sb[:, 0], xlo_sb[:, 0, bass.ts(s, 512)],
                             start=False, stop=False)
            nc.tensor.matmul(ps, wq_sb[:, 1], xb_sb[:, 1, bass.ts(s, 512)],
                             start=False, stop=False)
            nc.tensor.matmul(ps, wq_sb[:, 1], xlo_sb[:, 1, bass.ts(s, 512)],
                             start=False, stop=True)
            nc.scalar.activation(out=qq_sb[:, bass.ts(s, 512)], in_=ps,
                                 func=IDENT, bias=qb_sb, scale=1.0)

        def kk_slice(s):
            ps = pp.tile([P, 512], F32, tag="pp", name=f"kk_ps_{s}")
            nc.tensor.matmul(ps, wk_sb[:, 0], xb_sb[:, 0, bass.ts(s, 512)],
                             start=True, stop=False)
            nc.tensor.matmul(ps, wk_sb[:, 1], xb_sb[:, 1, bass.ts(s, 512)],
                             start=False, stop=True)
            nc.scalar.activation(out=kk_sb[:, bass.ts(s, 512)], in_=ps,
                                 func=IDENT, bias=kb_sb, scale=1.0)

        def vt_chunk(j):
            ps = pp.tile([P, C], F32, tag="pp", name=f"vt_ps_{j}")
            nc.tensor.matmul(ps, xb_sb[:, 0, bass.ts(j, P)], wv_sb[:, 0],
                             start=True, stop=False)
            nc.tensor.matmul(ps, xb_sb[:, 1, bass.ts(j, P)], wv_sb[:, 1],
                             start=False, stop=True)
            nc.vector.tensor_add(vt_sb[:, j], ps, vb_bc)

        # consume xb strictly in chunk-arrival order; the o=1 half of
        # chunk s+2 is issued from the scalar queue as chunk s is consumed
        # (chunks 6-7 ride the Pool queue instead); qq slices 1-3 are
        # emitted mid-phase, right before the pipeline needs them
        kk_slice(0)
        for j in range(4):
            vt_chunk(j)
        qq_slice(0)
        for s in range(1, N // 512):
            if s + 1 < 6:
                ld(nc.scalar, xb_sb, xb_d, 1, s + 1)
            kk_slice(s)
            for j in range(4 * s, 4 * s + 4):
                vt_chunk(j)
        qq_slice(1)
        qq_slice(2)
        qq_slice(3)

        # full residual r = xb + xlo (exact fp32 x), precomputed on Pool
        # (idle during projections) so each phase's finalize needs only one
        # add per channel half; chunked to follow the xlo DMA arrivals
        r_sb = singles.tile([P, 2, NI], F32)
        for c in range(4):
            nc.gpsimd.tensor_add(r_sb[:, :, bass.ts(c, 512)],
                                 xb_sb[:, :, bass.ts(c, 512)],
                                 xlo_sb[:, :, bass.ts(c, 512)])

        # ---- attention ----------------------------------------------------------
        # Row-packed QK: pair (jA, jB) = (2t, 2t+1); jA on PE rows 0-63, jB on
        # rows 64-127 (via the duplicated q/k halves), running concurrently.
        NPAIR = N // P // 2   # 16 pairs per phase
        NPH = NI // PH        # 4 phases

        def issue_pair(ph, t):
            # One PSUM tile holds both halves' S^T slices ([P, 2, 512]); the
            # two K=64 QK matmuls row-pack (rows 0-63 / 64-127) and a SINGLE
            # [128, 1024] exp covers both halves (amortizes ScalarE's fixed
            # per-instruction overhead -- ScalarE is the co-critical engine).
            i0 = ph * PH
            ps = p_s.tile([P, 2, PH], F32, tag="s", name=f"ps_{ph}_{t}")
            for h, j in ((0, 2 * t), (1, 2 * t + 1)):
                lo = h * CQ
                nc.tensor.matmul(
                    ps[:, h],
                    kk_sb[lo:lo + CQ, bass.ts(j, P)],
                    qq_sb[lo:lo + CQ, bass.ds(i0, PH)],
                    start=True, stop=True)
            et = etp.tile([P, 2, PH], BF16, tag="et", name=f"et_{ph}_{t}")
            nc.scalar.activation(out=et, in_=ps, func=EXP, scale=1.0)
            return et

        def pv_half(po, t, h, et):
            j = 2 * t + h
            for cc in range(C // P):
                nc.tensor.matmul(
                    po[cc],
                    vt_sb[:, j, bass.ts(cc, P)],
                    et[:, h],
                    start=(t == 0 and h == 0), stop=(t == NPAIR - 1 and h == 1))

        def finalize_z(ph, z2, et15=None):
            # Z colsum + partition-broadcast via all-ones bf16 lhsT matmuls
            # accumulated into one PSUM tile; emitted between the last
            # pair's two PV halves so the reciprocal overlaps the remaining
            # PV stream.  For the final phase the last pair's exp tile is
            # summed directly by the PE (et15) so the colsum can issue a
            # full pair earlier, pulling the reciprocal off the tail.
            pzb = p_s.tile([P, PH], F32, tag="s", name=f"pzb_{ph}")
            nc.tensor.matmul(pzb, ones_b, z2[:, 0], start=True, stop=False)
            nc.tensor.matmul(pzb, ones_b, z2[:, 1],
                             start=False, stop=et15 is None)
            if et15 is not None:
                nc.tensor.matmul(pzb, ones_b, et15[:, 0],
                                 start=False, stop=False)
                nc.tensor.matmul(pzb, ones_b, et15[:, 1],
                                 start=False, stop=True)
            zbc = zp.tile([P, PH], F32, tag="zbc", name=f"zbc_{ph}")
            nc.vector.reciprocal_approx_fast(out=zbc, in_=pzb)
            return zbc

        def finalize_out(ph, po, zbc):
            # ob = (po * gamma) * (1/Z) via one STT per channel half, then a
            # single +r residual add.  Middle phases put the adds on Pool
            # while the PE begins the next phase (po PSUM banks rotate
            # between phases); the last phase keeps them on Vector (shorter
            # tail).  DMA issues ride the sync queue and the engine that
            # produced the data -- NEVER the scalar queue, whose in-order
            # stream would stall the critical exps behind the DMA's wait.
            last = ph == NPH - 1
            sl_i = bass.ds(ph * PH, PH)
            ob = outp.tile([P, 2, PH], F32, tag="ob", name=f"ob_{ph}")
            for cc in range(C // P):
                nc.vector.scalar_tensor_tensor(
                    out=ob[:, cc], in0=po[cc], scalar=gamma_bc, in1=zbc,
                    op0=MUL, op1=MUL)
                if last:
                    nc.vector.tensor_add(ob[:, cc], ob[:, cc],
                                         r_sb[:, cc, sl_i])
                else:
                    nc.gpsimd.tensor_add(ob[:, cc], ob[:, cc],
                                         r_sb[:, cc, sl_i])
                q = nc.sync if cc == 0 else nc.gpsimd
                q.dma_start(out=out_d[:][:, cc:cc + 1, sl_i],
                            in_=ob[:, cc:cc + 1])

        # Software pipeline over double-pair steps: per step, issue QK/exp
        # for the NEXT two pairs back-to-back, then run all 8 PV matmuls of
        # the current two pairs.  Batching the QK groups halves the number of
        # QK<->PV transitions (each costs ~300-400ns of weight-load/drain
        # exposure), and the ~2.2us of exp still fits inside the step.
        all_pairs = [(ph, t) for ph in range(NPH) for t in range(NPAIR)]
        pend = {pr: issue_pair(*pr) for pr in all_pairs[:2]}
        z2_by_ph = {}
        po_by_ph = {}
        for i in range(0, len(all_pairs), 2):
            pra, prb = all_pairs[i], all_pairs[i + 1]
            ph = pra[0]
            if pra[1] == 0:
                z2_by_ph[ph] = zp.tile([P, 2, PH], BF16, tag="z2",
                                       name=f"z2_{ph}")
                po_by_ph[ph] = [pp.tile([P, PH], F32, tag="pp",
                                        name=f"po_{ph}_{cc}")
                                for cc in range(C // P)]
            z2 = z2_by_ph[ph]
            po = po_by_ph[ph]
            eta, etb = pend.pop(pra), pend.pop(prb)
            for pr in all_pairs[i + 2:i + 4]:
                pend[pr] = issue_pair(*pr)
            last_ph = ph == NPH - 1
            for t, et in ((pra[1], eta), (prb[1], etb)):
                if t == 0:
                    nc.vector.tensor_copy(z2, et)
                elif not (last_ph and t == NPAIR - 1):
                    nc.vector.tensor_add(z2, z2, et)
                if t == NPAIR - 1:
                    if last_ph:
                        # 4-term colsum before pair 15's PVs: the reciprocal
                        # and the finalize chain leave the kernel tail
                        zbc = finalize_z(ph, z2, et15=et)
                        pv_half(po, t, 0, et)
                        pv_half(po, t, 1, et)
                    else:
                        pv_half(po, t, 0, et)
                        zbc = finalize_z(ph, z2)
                        pv_half(po, t, 1, et)
                    finalize_out(ph, po, zbc)
                else:
                    pv_half(po, t, 0, et)
                    pv_half(po, t, 1, et)


def _build_nc():
    nc = bacc.Bacc(trn_type="TRN2", target_bir_lowering=False, debug=False)
    aps = (
        nc.declare_dram_parameter("xb", [P, 2, N], BF16, isOutput=False),
        nc.declare_dram_parameter("xlo", [P, 2, NI], BF16, isOutput=False),
        nc.declare_dram_parameter("wq", [P, 2, P], BF16, isOutput=False),
        nc.declare_dram_parameter("wk", [P, 2, P], BF16, isOutput=False),
        nc.declare_dram_parameter("wv", [P, 2, C], BF16, isOutput=False),
        nc.declare_dram_parameter("qb", [P, 1], F32, isOutput=False),
        nc.declare_dram_parameter("kb", [P, 1], F32, isOutput=False),
        nc.declare_dram_parameter("vb_bc", [P, C], F32, isOutput=False),
        nc.declare_dram_parameter("gamma_bc", [P, 1], F32, isOutput=False),
        nc.declare_dram_parameter("out", [P, 2, NI], F32, isOutput=True),
    )
    with tile.TileContext(nc) as tc:
        _emit(tc, aps)
    nc.compile()
    return nc


_NC_CACHE = {}


def get_nc():
    if "nc" not in _NC_CACHE:
        _NC_CACHE["nc"] = _build_nc()
    return _NC_CACHE["nc"]


def _pmajor(a, free):
    """[256, free] -> contiguous [128, 2, free] partition-major view."""
    return np.ascontiguousarray(a.reshape(2, P, free).transpose(1, 0, 2))


def make_in_maps(inputs):
    """Build the 8 per-core input maps from the full problem inputs."""
    import ml_dtypes
    f = np.float32
    bf = ml_dtypes.bfloat16
    x_streams = [
        np.ascontiguousarray(inputs["input1"].reshape(B, C, N), dtype=f),
        np.ascontiguousarray(inputs["input2"].reshape(B, C, N), dtype=f),
    ]
    wsets = []
    for s in ("1", "2"):
        qw = np.asarray(inputs[f"q{s}_w"], dtype=f)
        kw = np.asarray(inputs[f"k{s}_w"], dtype=f)
        vw = np.asarray(inputs[f"v{s}_w"], dtype=f)
        qb = np.asarray(inputs[f"q{s}_b"], dtype=f)
        kb = np.asarray(inputs[f"k{s}_b"], dtype=f)
        vb = np.asarray(inputs[f"v{s}_b"], dtype=f)
        wsets.append(dict(
            wq=_pmajor(np.concatenate([qw, qw], 0).T.astype(bf), P),
            wk=_pmajor(np.concatenate([kw, kw], 0).T.astype(bf), P),
            wv=_pmajor(vw.T.astype(bf), C),
            qb=np.ascontiguousarray(np.concatenate([qb, qb])[:, None]),
            kb=np.ascontiguousarray(np.concatenate([kb, kb])[:, None]),
            vb_bc=np.ascontiguousarray(np.broadcast_to(vb[None, :], (P, C)),
                                       dtype=f),
        ))
    gamma = np.asarray(inputs["gamma"], dtype=f).reshape(1, 1)
    gamma_bc = np.ascontiguousarray(np.broadcast_to(gamma, (P, 1)))

    in_maps = []
    for core in range(8):
        u, h = core // 2, core % 2
        b, s = u // 2, u % 2
        xs = x_streams[s][b]
        # rotate so this core's 2048 query columns come first (attention
        # contracts over all keys, so key/value column order is irrelevant)
        xrot = np.concatenate([xs[:, h * NI:], xs[:, :h * NI]], axis=1)
        xb = xrot.astype(bf)
        xlo = (xrot[:, :NI] - xb[:, :NI].astype(f)).astype(bf)
        m = dict(wsets[s])
        m["xb"] = _pmajor(xb, N)
        m["xlo"] = _pmajor(xlo, NI)
        m["gamma_bc"] = gamma_bc
        in_maps.append(m)
    return in_maps


def assemble(results, inputs):
    """Stitch the 8 per-core [128, 2, 2048] outputs into (out1, out2)."""
    outs = [np.empty((B, C, N), np.float32) for _ in range(2)]
    for core in range(8):
        u, h = core // 2, core % 2
        b, s = u // 2, u % 2
        o = results[core]["out"].transpose(1, 0, 2).reshape(C, NI)
        outs[s][b][:, h * NI:(h + 1) * NI] = o
    out1 = outs[0].reshape(B, C, H, W)
    out2 = outs[1].reshape(B, C, H, W)
    return out1, out2


def kernel(**inputs):
    from concourse.bass_utils import run_bass_kernel_spmd

    nc = get_nc()
    in_maps = make_in_maps(inputs)
    res = run_bass_kernel_spmd(nc, in_maps, list(range(8)))
    return assemble(res.results, inputs)


# revision 30
# speedup vs baseline: 1.1138x; 1.0209x over previous
"""Trainium2 Bass kernel for nn_AttShare: dual-stream 1x1-conv attention.

Full-input contract: kernel(**inputs) takes the complete tensors from
setup_inputs() and returns (out1, out2) exactly like the reference.

Sharding (8 cores): 4 independent (batch, stream) attention units x 2-way
query-row split.  Each core gets the full x=[256,4096] of its unit, HOST-
ROTATED so its 2048 query columns come first; it produces
out = gamma * (V @ softmax(Q K^T)^T)[:, 0:2048] + x[:, 0:2048].
(Attention contracts over all keys, so the key/value column order is
irrelevant; the host scatters the output back to the right columns.)

Key simplification: the reference adds a per-row bias (q . g) to the logits
before a row-softmax.  softmax is shift-invariant per row, so the entire
global-gating branch (pooled means -> MLP -> sigmoid -> bias) cancels and is
not computed.  The k-projection bias also shifts logits uniformly per row
and cancels; the q bias does not and is applied.  The v bias adds
gamma*vb[c] (softmax rows sum to 1); it is folded into the V^T tiles;
gamma itself is folded into the finalize scalar_tensor_tensor (exact).

Precision: everything rides bf16 except the PSUM accumulations (f32) and
the residual, which is reconstructed exactly from a bf16 hi+lo split of x
(xb + xlo == fp32 x to ~2^-17).  The q projection also uses the hi+lo
split (two bf16 matmul passes == fp32r's 2 cycles/col, but no fp32 DMA);
k/v project from xb alone.  Measured accuracy ~1.25e-2 relative
(tolerance 2e-2); the dominant term is the bf16 rounding of q/k.

On-core dataflow (per core):
  proj:  kk = Wk_dup @ xb (+kb)            [128, 4096] -> bf16 (k dup'd on
         qq = Wq_dup @ (xb+xlo) (+qb)      [128, 2048] -> bf16  both halves
         vt = xb^T Wv^T (+vb via Vector)   [128 j, 32, 256] bf16 for packing)
  attn (4 phases of 512 query columns, keys streamed in row-packed pairs
        of 128-key chunks, software-pipelined in double-pair steps):
         S^T = kk_j^T @ qq  (K=64, PE rows 0-63 / 64-127 run CONCURRENTLY
               via row tiling; both pairs' QK groups issued back-to-back
               to halve the QK<->PV weight-load transitions)
         E = exp(S^T)  (one [128,1024] ScalarE pass per pair covering both
               halves; PSUM -> bf16 SBUF; no max-shift needed: |S|<~60 and
               the denominator normalizes later)
         z2 += E  (Vector, one [128,1024] bf16 add per pair, incl. pair 15)
         out_psum[c,i] += vt_j^T @ E  (bf16 matmuls, PSUM-resident)
  finalize per phase: Z colsum+broadcast via 2 all-ones bf16 lhsT matmuls
  (emitted between the last pair's PV halves), fast reciprocal (Vector,
  18-bit), out = (po * gamma) * recip + xb + xlo, DMA out split across
  both rings by channel half.  Middle phases put the +x adds on Pool; the
  last phase adds a Pool-precomputed r_last = xb+xlo on Vector (short
  tail).  PSUM budget 8 banks: 2x2 phase-rotated output accumulators +
  2x2-bank S^T slots, so a phase's finalize never blocks the next phase's
  PV.

Head: all tensors are pre-permuted on host to partition-major [128, 2, n]
bf16 so every DMA is a dense descriptor; the two cin-halves ride the SP
and Activation hardware DMA rings (~150GB/s each); xlo chunks 1-3 trail
the queue (their qq slices are consumed last).
"""

import os
import sys

import numpy as np

for _p in ("/opt/trn_rl_repo", os.path.expanduser("~/.axon_site/_ro/trn_rl_repo")):
    if os.path.isdir(_p) and _p not in sys.path:
        sys.path.insert(0, _p)

import concourse.bass as bass  # noqa: E402
import concourse.bacc as bacc  # noqa: E402
import concourse.mybir as mybir  # noqa: E402
import concourse.tile as tile  # noqa: E402

P = 128
C = 256         # channels
CQ = 64         # q/k channels
N = 4096        # H*W
NI = 2048       # query rows per core
PH = 512        # query columns processed per phase
B, H, W = 2, 64, 64
F32 = mybir.dt.float32
BF16 = mybir.dt.bfloat16


def _emit(tc, aps):
    nc = tc.nc
    import contextlib

    (xb_d, xlo_d, wq_d, wk_d, wv_d, qb_d, kb_d, vbb_d, gbc_d, out_d) = aps
    EXP = mybir.ActivationFunctionType.Exp
    IDENT = mybir.ActivationFunctionType.Identity
    MUL = mybir.AluOpType.mult
    ADD = mybir.AluOpType.add

    with contextlib.ExitStack() as ctx:
        singles = ctx.enter_context(tc.tile_pool(name="singles", bufs=1))
        pp = ctx.enter_context(tc.tile_pool(name="pp", bufs=4, space="PSUM"))
        p_s = ctx.enter_context(tc.tile_pool(name="p_s", bufs=2, space="PSUM"))
        etp = ctx.enter_context(tc.tile_pool(name="etp", bufs=8))
        zp = ctx.enter_context(tc.tile_pool(name="zp", bufs=4))
        outp = ctx.enter_context(tc.tile_pool(name="outp", bufs=4))

        # ---- loads --------------------------------------------------------------
        xb_sb = singles.tile([P, 2, N], BF16)     # full rotated x, bf16 hi
        xlo_sb = singles.tile([P, 2, NI], BF16)   # query-half lo correction
        wq_sb = singles.tile([P, 2, P], BF16)
        wk_sb = singles.tile([P, 2, P], BF16)
        wv_sb = singles.tile([P, 2, C], BF16)

        kb_sb = singles.tile([P, 1], F32)
        qb_sb = singles.tile([P, 1], F32)
        gamma_bc = singles.tile([P, 1], F32)   # host pre-broadcast
        vb_bc = singles.tile([P, C], F32)      # host pre-broadcast

        def ld(queue, sb, dram, o, c):
            queue.dma_start(out=sb[:, o:o + 1, bass.ts(c, 512)],
                            in_=dram[:][:, o:o + 1, bass.ts(c, 512)])

        def ld2(queue, sb, dram, c):
            queue.dma_start(out=sb[:, :, bass.ts(c, 512)],
                            in_=dram[:][:, :, bass.ts(c, 512)])

        # Three-queue head: SP carries o=0 x halves (issue-only engine,
        # front-loaded); Activation carries o=1 halves, paced inside the
        # projection emission so descriptor writes never starve the
        # identity activations; Pool carries the tiny params, both
        # pre-broadcasts, xlo, and the last two x chunks.  ~1.1MB/queue.
        nc.gpsimd.dma_start(out=qb_sb, in_=qb_d[:])
        nc.gpsimd.dma_start(out=kb_sb, in_=kb_d[:])
        nc.gpsimd.dma_start(out=gamma_bc, in_=gbc_d[:])
        nc.gpsimd.dma_start(out=vb_bc, in_=vbb_d[:])
        nc.gpsimd.dma_start(out=xlo_sb[:, :, bass.ds(0, 512)],
                            in_=xlo_d[:][:, :, bass.ds(0, 512)])
        ld2(nc.gpsimd, xb_sb, xb_d, 6)
        ld2(nc.gpsimd, xb_sb, xb_d, 7)
        nc.gpsimd.dma_start(out=xlo_sb[:, :, bass.ds(512, 1536)],
                            in_=xlo_d[:][:, :, bass.ds(512, 1536)])

        nc.sync.dma_start(out=wk_sb, in_=wk_d[:])
        nc.sync.dma_start(out=wq_sb, in_=wq_d[:])
        for c in range(6):
            ld(nc.sync, xb_sb, xb_d, 0, c)
        nc.scalar.dma_start(out=wv_sb, in_=wv_d[:])
        ld(nc.scalar, xb_sb, xb_d, 1, 0)
        ld(nc.scalar, xb_sb, xb_d, 1, 1)

        ones_b = singles.tile([P, P], BF16)   # all-ones bf16 lhsT: Z colsum
        nc.vector.memset(ones_b, 1.0)

        # ---- projections --------------------------------------------------------
        # qq/kk stored bf16: the QK matmuls then stream 1 cycle/col with
        # single-pass weight loads.  All projection matmuls are bf16 (fp32r
        # runs 2 cycles/col on HW); q keeps full x precision via the hi+lo
        # split (two accumulating bf16 passes).
        qq_sb = singles.tile([P, NI], BF16)    # [q; q] duplicated across halves
        kk_sb = singles.tile([P, N], BF16)     # [k; k] duplicated across halves
        vt_sb = singles.tile([P, N // P, C], BF16)   # V^T: [j, c], +vb folded

        def qq_slice(s):
            ps = pp.tile([P, 512], F32, tag="pp", name=f"qq_ps_{s}")
            nc.tensor.matmul(ps, wq_sb[:, 0], xb_sb[:, 0, bass.ts(s, 512)],
                             start=True, stop=False)
            nc.tensor.matmul(ps, wq_sb[:, 0], xlo_sb[:, 0, bass.ts(s, 512)],
                             start=False, stop=False)
            nc.tensor.matmul(ps, wq_sb[:, 1], xb_sb[:, 1, bass.ts(s, 512)],
                             start=False, stop=False)
            nc.tensor.matmul(ps, wq_sb[:, 1], xlo_sb[:, 1, bass.ts(s, 512)],
                             start=False, stop=True)
            nc.scalar.activation(out=qq_sb[:, bass.ts(s, 512)], in_=ps,
                                 func=IDENT, bias=qb_sb, scale=1.0)

        def kk_slice(s):
            ps = pp.tile([P, 512], F32, tag="pp", name=f"kk_ps_{s}")
            nc.tensor.matmul(ps, wk_sb[:, 0], xb_sb[:, 0, bass.ts(s, 512)],
                             start=True, stop=False)
            nc.tensor.matmul(ps, wk_sb[:, 1], xb_sb[:, 1, bass.ts(s, 512)],
                             start=False, stop=True)
            nc.scalar.activation(out=kk_sb[:, bass.ts(s, 512)], in_=ps,
                                 func=IDENT, bias=kb_sb, scale=1.0)

        def vt_chunk(j):
            ps = pp.tile([P, C], F32, tag="pp", name=f"vt_ps_{j}")
            nc.tensor.matmul(ps, xb_sb[:, 0, bass.ts(j, P)], wv_sb[:, 0],
                             start=True, stop=False)
            nc.tensor.matmul(ps, xb_sb[:, 1, bass.ts(j, P)], wv_sb[:, 1],
                             start=False, stop=True)
            nc.vector.tensor_add(vt_sb[:, j], ps, vb_bc)

        # consume xb strictly in chunk-arrival order; the o=1 half of
        # chunk s+2 is issued from the scalar queue as chunk s is consumed
        # (chunks 6-7 ride the Pool queue instead); qq slices 1-3 are
        # emitted mid-phase, right before the pipeline needs them
        kk_slice(0)
        for j in range(4):
            vt_chunk(j)
        qq_slice(0)
        for s in range(1, N // 512):
            if s + 1 < 6:
                ld(nc.scalar, xb_sb, xb_d, 1, s + 1)
            kk_slice(s)
            for j in range(4 * s, 4 * s + 4):
                vt_chunk(j)
        qq_slice(1)
        qq_slice(2)
        qq_slice(3)

        # full residual r = xb + xlo (exact fp32 x), precomputed on Pool
        # (idle during projections) so each phase's finalize needs only one
        # add per channel half; chunked to follow the xlo DMA arrivals
        r_sb = singles.tile([P, 2, NI], F32)
        for c in range(4):
            nc.gpsimd.tensor_add(r_sb[:, :, bass.ts(c, 512)],
                                 xb_sb[:, :, bass.ts(c, 512)],
                                 xlo_sb[:, :, bass.ts(c, 512)])

        # ---- attention ----------------------------------------------------------
        # Row-packed QK: pair (jA, jB) = (2t, 2t+1); jA on PE rows 0-63, jB on
        # rows 64-127 (via the duplicated q/k halves), running concurrently.
        NPAIR = N // P // 2   # 16 pairs per phase
        NPH = NI // PH        # 4 phases

        def issue_pair(ph, t):
            # One PSUM tile holds both halves' S^T slices ([P, 2, 512]); the
            # two K=64 QK matmuls row-pack (rows 0-63 / 64-127) and a SINGLE
            # [128, 1024] exp covers both halves (amortizes ScalarE's fixed
            # per-instruction overhead -- ScalarE is the co-critical engine).
            i0 = ph * PH
            ps = p_s.tile([P, 2, PH], F32, tag="s", name=f"ps_{ph}_{t}")
            for h, j in ((0, 2 * t), (1, 2 * t + 1)):
                lo = h * CQ
                nc.tensor.matmul(
                    ps[:, h],
                    kk_sb[lo:lo + CQ, bass.ts(j, P)],
                    qq_sb[lo:lo + CQ, bass.ds(i0, PH)],
                    start=True, stop=True)
            et = etp.tile([P, 2, PH], BF16, tag="et", name=f"et_{ph}_{t}")
            nc.scalar.activation(out=et, in_=ps, func=EXP, scale=1.0)
            return et

        def pv_half(po, t, h, et):
            j = 2 * t + h
            for cc in range(C // P):
                nc.tensor.matmul(
                    po[cc],
                    vt_sb[:, j, bass.ts(cc, P)],
                    et[:, h],
                    start=(t == 0 and h == 0), stop=(t == NPAIR - 1 and h == 1))

        def finalize_z(ph, z2, et15=None):
            # Z colsum + partition-broadcast via all-ones bf16 lhsT matmuls
            # accumulated into one PSUM tile; emitted between the last
            # pair's two PV halves so the reciprocal overlaps the remaining
            # PV stream.  For the final phase the last pair's exp tile is
            # summed directly by the PE (et15) so the colsum can issue a
            # full pair earlier, pulling the reciprocal off the tail.
            pzb = p_s.tile([P, PH], F32, tag="s", name=f"pzb_{ph}")
            nc.tensor.matmul(pzb, ones_b, z2[:, 0], start=True, stop=False)
            nc.tensor.matmul(pzb, ones_b, z2[:, 1],
                             start=False, stop=et15 is None)
            if et15 is not None:
                nc.tensor.matmul(pzb, ones_b, et15[:, 0],
                                 start=False, stop=False)
                nc.tensor.matmul(pzb, ones_b, et15[:, 1],
                                 start=False, stop=True)
            zbc = zp.tile([P, PH], F32, tag="zbc", name=f"zbc_{ph}")
            nc.vector.reciprocal_approx_fast(out=zbc, in_=pzb)
            return zbc

        def finalize_out(ph, po, zbc):
            # ob = (po * gamma) * (1/Z) via one STT per channel half, then a
            # single +r residual add.  Middle phases put the adds on Pool
            # while the PE begins the next phase (po PSUM banks rotate
            # between phases); the last phase keeps them on Vector (shorter
            # tail).  DMA issues ride the sync queue and the engine that
            # produced the data -- NEVER the scalar queue, whose in-order
            # stream would stall the critical exps behind the DMA's wait.
            last = ph == NPH - 1
            sl_i = bass.ds(ph * PH, PH)
            ob = outp.tile([P, 2, PH], F32, tag="ob", name=f"ob_{ph}")
            ob2 = outp.tile([P, 2, PH], BF16, tag="ob2", name=f"ob2_{ph}")
            for cc in range(C // P):
                nc.vector.scalar_tensor_tensor(
                    out=ob[:, cc], in0=po[cc], scalar=gamma_bc, in1=zbc,
                    op0=MUL, op1=MUL)
                # single bf16 rounding at the very end (out ships bf16,
                # halving the output DMA; host upcasts)
                if last:
                    nc.vector.tensor_add(ob2[:, cc], ob[:, cc],
                                         r_sb[:, cc, sl_i])
                else:
                    nc.gpsimd.tensor_add(ob2[:, cc], ob[:, cc],
                                         r_sb[:, cc, sl_i])
                if cc == 0:
                    q = nc.sync
                else:
                    q = nc.scalar if last else nc.gpsimd
                q.dma_start(out=out_d[:][:, cc:cc + 1, sl_i],
                            in_=ob2[:, cc:cc + 1])

        # Software pipeline over double-pair steps: per step, issue QK/exp
        # for the NEXT two pairs back-to-back, then run all 8 PV matmuls of
        # the current two pairs.  Batching the QK groups halves the number of
        # QK<->PV transitions (each costs ~300-400ns of weight-load/drain
        # exposure), and the ~2.2us of exp still fits inside the step.
        all_pairs = [(ph, t) for ph in range(NPH) for t in range(NPAIR)]
        pend = {pr: issue_pair(*pr) for pr in all_pairs[:2]}
        z2_by_ph = {}
        po_by_ph = {}
        for i in range(0, len(all_pairs), 2):
            pra, prb = all_pairs[i], all_pairs[i + 1]
            ph = pra[0]
            if pra[1] == 0:
                z2_by_ph[ph] = zp.tile([P, 2, PH], BF16, tag="z2",
                                       name=f"z2_{ph}")
                po_by_ph[ph] = [pp.tile([P, PH], F32, tag="pp",
                                        name=f"po_{ph}_{cc}")
                                for cc in range(C // P)]
            z2 = z2_by_ph[ph]
            po = po_by_ph[ph]
            eta, etb = pend.pop(pra), pend.pop(prb)
            for pr in all_pairs[i + 2:i + 4]:
                pend[pr] = issue_pair(*pr)
            last_ph = ph == NPH - 1
            for t, et in ((pra[1], eta), (prb[1], etb)):
                if t == 0:
                    nc.vector.tensor_copy(z2, et)
                elif not (last_ph and t == NPAIR - 1):
                    nc.vector.tensor_add(z2, z2, et)
                if t == NPAIR - 1:
                    if last_ph:
                        # 4-term colsum before pair 15's PVs: the reciprocal
                        # and the finalize chain leave the kernel tail
                        zbc = finalize_z(ph, z2, et15=et)
                        pv_half(po, t, 0, et)
                        pv_half(po, t, 1, et)
                    else:
                        pv_half(po, t, 0, et)
                        zbc = finalize_z(ph, z2)
                        pv_half(po, t, 1, et)
                    finalize_out(ph, po, zbc)
                else:
                    pv_half(po, t, 0, et)
                    pv_half(po, t, 1, et)


def _build_nc():
    nc = bacc.Bacc(trn_type="TRN2", target_bir_lowering=False, debug=False)
    aps = (
        nc.declare_dram_parameter("xb", [P, 2, N], BF16, isOutput=False),
        nc.declare_dram_parameter("xlo", [P, 2, NI], BF16, isOutput=False),
        nc.declare_dram_parameter("wq", [P, 2, P], BF16, isOutput=False),
        nc.declare_dram_parameter("wk", [P, 2, P], BF16, isOutput=False),
        nc.declare_dram_parameter("wv", [P, 2, C], BF16, isOutput=False),
        nc.declare_dram_parameter("qb", [P, 1], F32, isOutput=False),
        nc.declare_dram_parameter("kb", [P, 1], F32, isOutput=False),
        nc.declare_dram_parameter("vb_bc", [P, C], F32, isOutput=False),
        nc.declare_dram_parameter("gamma_bc", [P, 1], F32, isOutput=False),
        nc.declare_dram_parameter("out", [P, 2, NI], BF16, isOutput=True),
    )
    with tile.TileContext(nc) as tc:
        _emit(tc, aps)
    nc.compile()
    return nc


_NC_CACHE = {}


def get_nc():
    if "nc" not in _NC_CACHE:
        _NC_CACHE["nc"] = _build_nc()
    return _NC_CACHE["nc"]


def _pmajor(a, free):
    """[256, free] -> contiguous [128, 2, free] partition-major view."""
    return np.ascontiguousarray(a.reshape(2, P, free).transpose(1, 0, 2))


def make_in_maps(inputs):
    """Build the 8 per-core input maps from the full problem inputs."""
    import ml_dtypes
    f = np.float32
    bf = ml_dtypes.bfloat16
    x_streams = [
        np.ascontiguousarray(inputs["input1"].reshape(B, C, N), dtype=f),
        np.ascontiguousarray(inputs["input2"].reshape(B, C, N), dtype=f),
    ]
    wsets = []
    for s in ("1", "2"):
        qw = np.asarray(inputs[f"q{s}_w"], dtype=f)
        kw = np.asarray(inputs[f"k{s}_w"], dtype=f)
        vw = np.asarray(inputs[f"v{s}_w"], dtype=f)
        qb = np.asarray(inputs[f"q{s}_b"], dtype=f)
        kb = np.asarray(inputs[f"k{s}_b"], dtype=f)
        vb = np.asarray(inputs[f"v{s}_b"], dtype=f)
        wsets.append(dict(
            wq=_pmajor(np.concatenate([qw, qw], 0).T.astype(bf), P),
            wk=_pmajor(np.concatenate([kw, kw], 0).T.astype(bf), P),
            wv=_pmajor(vw.T.astype(bf), C),
            qb=np.ascontiguousarray(np.concatenate([qb, qb])[:, None]),
            kb=np.ascontiguousarray(np.concatenate([kb, kb])[:, None]),
            vb_bc=np.ascontiguousarray(np.broadcast_to(vb[None, :], (P, C)),
                                       dtype=f),
        ))
    gamma = np.asarray(inputs["gamma"], dtype=f).reshape(1, 1)
    gamma_bc = np.ascontiguousarray(np.broadcast_to(gamma, (P, 1)))

    in_maps = []
    for core in range(8):
        u, h = core // 2, core % 2
        b, s = u // 2, u % 2
        xs = x_streams[s][b]
        # rotate so this core's 2048 query columns come first (attention
        # contracts over all keys, so key/value column order is irrelevant)
        xrot = np.concatenate([xs[:, h * NI:], xs[:, :h * NI]], axis=1)
        xb = xrot.astype(bf)
        xlo = (xrot[:, :NI] - xb[:, :NI].astype(f)).astype(bf)
        m = dict(wsets[s])
        m["xb"] = _pmajor(xb, N)
        m["xlo"] = _pmajor(xlo, NI)
        m["gamma_bc"] = gamma_bc
        in_maps.append(m)
    return in_maps


def assemble(results, inputs):
    """Stitch the 8 per-core [128, 2, 2048] outputs into (out1, out2)."""
    outs = [np.empty((B, C, N), np.float32) for _ in range(2)]
    for core in range(8):
        u, h = core // 2, core % 2
        b, s = u // 2, u % 2
        o = results[core]["out"].astype(np.float32).transpose(1, 0, 2).reshape(C, NI)
        outs[s][b][:, h * NI:(h + 1) * NI] = o
    out1 = outs[0].reshape(B, C, H, W)
    out2 = outs[1].reshape(B, C, H, W)
    return out1, out2


def kernel(**inputs):
    from concourse.bass_utils import run_bass_kernel_spmd

    nc = get_nc()
    in_maps = make_in_maps(inputs)
    res = run_bass_kernel_spmd(nc, in_maps, list(range(8)))
    return assemble(res.results, inputs)


# revision 31
# speedup vs baseline: 1.1144x; 1.0005x over previous
"""Trainium2 Bass kernel for nn_AttShare: dual-stream 1x1-conv attention.

Full-input contract: kernel(**inputs) takes the complete tensors from
setup_inputs() and returns (out1, out2) exactly like the reference.

Sharding (8 cores): 4 independent (batch, stream) attention units x 2-way
query-row split.  Each core gets the full x=[256,4096] of its unit, HOST-
ROTATED so its 2048 query columns come first; it produces
out = gamma * (V @ softmax(Q K^T)^T)[:, 0:2048] + x[:, 0:2048].
(Attention contracts over all keys, so the key/value column order is
irrelevant; the host scatters the output back to the right columns.)

Key simplification: the reference adds a per-row bias (q . g) to the logits
before a row-softmax.  softmax is shift-invariant per row, so the entire
global-gating branch (pooled means -> MLP -> sigmoid -> bias) cancels and is
not computed.  The k-projection bias also shifts logits uniformly per row
and cancels; the q bias does not and is applied.  The v bias is folded
into the V^T tiles; gamma is folded into the finalize
scalar_tensor_tensor (exact); gamma and vb ship host-pre-broadcast so the
kernel needs no cold-start K=1 broadcast matmuls.

Precision: everything rides bf16 except the PSUM accumulations (f32) and
the residual, which is reconstructed exactly from a bf16 hi+lo split of x
(xb + xlo == fp32 x to ~2^-17).  The q projection also uses the hi+lo
split (two bf16 matmul passes == fp32r's measured 2 cycles/col, but no
fp32 DMA); k/v project from xb alone.  The output ships bf16 (single
final rounding) and the host upcasts.  Measured accuracy 1.267e-2
relative (tolerance 2e-2); the dominant term is bf16 rounding of q/k.

On-core dataflow (per core):
  proj:  kk = Wk_dup @ xb (+kb)            [128, 4096] -> bf16 (k dup'd on
         qq = Wq_dup @ (xb+xlo) (+qb)      [128, 2048] -> bf16  both halves
         vt = xb^T Wv^T (+vb via Vector)   [128 j, 32, 256] bf16 for packing)
       (Tile's dataflow scheduler automatically defers late-chunk proj
        work into attention phase 0 as the DMA arrives.)
  attn (4 phases of 512 query columns, keys streamed in row-packed pairs
        of 128-key chunks, software-pipelined in double-pair steps):
         S^T = kk_j^T @ qq  (K=64, PE rows 0-63 / 64-127 run CONCURRENTLY
               via row tiling -- a pair costs ~213ns, not 426; both pairs'
               QK groups issued back-to-back to halve the QK<->PV
               weight-load transitions)
         E = exp(S^T)  (one [128,1024] ScalarE pass per pair covering both
               halves; ScalarE is co-critical with the PE at ~18us/phase;
               no max-shift needed: |S|<~60 in bf16 and the denominator
               normalizes later)
         z2 += E  (Vector, one [128,1024] bf16 add per pair)
         out_psum[c,i] += vt_j^T @ E  (bf16 matmuls, PSUM-resident)
  finalize per phase: Z colsum+broadcast via all-ones bf16 lhsT matmuls
  (middle phases: 2 matmuls between the last pair's PV halves; final
  phase: 4 matmuls incl. the last exp tile directly, emitted BEFORE the
  last PVs so the reciprocal+STT chain leaves the kernel tail), fast
  reciprocal (Vector, 18-bit), ob = (po*gamma)*recip via STT, +r residual
  (r = xb+xlo precomputed on Pool), single bf16 rounding, DMA out.
  Middle phases put the +r adds on Pool while the PE starts the next
  phase; DMA issues ride sync/pool queues (and scalar only at the very
  end, once no exps remain) -- a DMA wait on the scalar queue mid-kernel
  would stall the critical exps behind it.
  PSUM budget 8 banks: 2x2 phase-rotated output accumulators + 2x2-bank
  S^T slots shared with the projection psums and Z colsums.

Head: all tensors are pre-permuted on host to partition-major [128, 2, n]
bf16 so every DMA is a dense descriptor.  Input bytes are spread over all
three DMA queues (~1.1MB each at ~130GB/s): SP carries x o=0 halves
(issue-only engine, front-loaded), Activation carries o=1 halves with the
descriptor writes paced inside the projection emission, Pool carries the
tiny params, pre-broadcasts, xlo, and the last two x chunks.
"""

import os
import sys

import numpy as np

for _p in ("/opt/trn_rl_repo", os.path.expanduser("~/.axon_site/_ro/trn_rl_repo")):
    if os.path.isdir(_p) and _p not in sys.path:
        sys.path.insert(0, _p)

import concourse.bass as bass  # noqa: E402
import concourse.bacc as bacc  # noqa: E402
import concourse.mybir as mybir  # noqa: E402
import concourse.tile as tile  # noqa: E402

P = 128
C = 256         # channels
CQ = 64         # q/k channels
N = 4096        # H*W
NI = 2048       # query rows per core
PH = 512        # query columns processed per phase
B, H, W = 2, 64, 64
F32 = mybir.dt.float32
BF16 = mybir.dt.bfloat16


def _emit(tc, aps):
    nc = tc.nc
    import contextlib

    (xb_d, xlo_d, wq_d, wk_d, wv_d, qb_d, kb_d, vbb_d, gbc_d, out_d) = aps
    EXP = mybir.ActivationFunctionType.Exp
    IDENT = mybir.ActivationFunctionType.Identity
    MUL = mybir.AluOpType.mult
    ADD = mybir.AluOpType.add

    with contextlib.ExitStack() as ctx:
        singles = ctx.enter_context(tc.tile_pool(name="singles", bufs=1))
        pp = ctx.enter_context(tc.tile_pool(name="pp", bufs=4, space="PSUM"))
        p_s = ctx.enter_context(tc.tile_pool(name="p_s", bufs=2, space="PSUM"))
        etp = ctx.enter_context(tc.tile_pool(name="etp", bufs=8))
        zp = ctx.enter_context(tc.tile_pool(name="zp", bufs=4))
        outp = ctx.enter_context(tc.tile_pool(name="outp", bufs=4))

        # ---- loads --------------------------------------------------------------
        xb_sb = singles.tile([P, 2, N], BF16)     # full rotated x, bf16 hi
        xlo_sb = singles.tile([P, 2, NI], BF16)   # query-half lo correction
        wq_sb = singles.tile([P, 2, P], BF16)
        wk_sb = singles.tile([P, 2, P], BF16)
        wv_sb = singles.tile([P, 2, C], BF16)

        kb_sb = singles.tile([P, 1], F32)
        qb_sb = singles.tile([P, 1], F32)
        gamma_bc = singles.tile([P, 1], F32)   # host pre-broadcast
        vb_bc = singles.tile([P, C], F32)      # host pre-broadcast

        def ld(queue, sb, dram, o, c):
            queue.dma_start(out=sb[:, o:o + 1, bass.ts(c, 512)],
                            in_=dram[:][:, o:o + 1, bass.ts(c, 512)])

        def ld2(queue, sb, dram, c):
            queue.dma_start(out=sb[:, :, bass.ts(c, 512)],
                            in_=dram[:][:, :, bass.ts(c, 512)])

        # Three-queue head: SP carries o=0 x halves (issue-only engine,
        # front-loaded); Activation carries o=1 halves, paced inside the
        # projection emission so descriptor writes never starve the
        # identity activations; Pool carries the tiny params, both
        # pre-broadcasts, xlo, and the last two x chunks.  ~1.1MB/queue.
        nc.gpsimd.dma_start(out=qb_sb, in_=qb_d[:])
        nc.gpsimd.dma_start(out=kb_sb, in_=kb_d[:])
        nc.gpsimd.dma_start(out=gamma_bc, in_=gbc_d[:])
        nc.gpsimd.dma_start(out=vb_bc, in_=vbb_d[:])
        nc.gpsimd.dma_start(out=xlo_sb[:, :, bass.ds(0, 512)],
                            in_=xlo_d[:][:, :, bass.ds(0, 512)])
        ld2(nc.gpsimd, xb_sb, xb_d, 6)
        ld2(nc.gpsimd, xb_sb, xb_d, 7)
        nc.gpsimd.dma_start(out=xlo_sb[:, :, bass.ds(512, 1536)],
                            in_=xlo_d[:][:, :, bass.ds(512, 1536)])

        nc.sync.dma_start(out=wk_sb, in_=wk_d[:])
        nc.sync.dma_start(out=wq_sb, in_=wq_d[:])
        for c in range(6):
            ld(nc.sync, xb_sb, xb_d, 0, c)
        nc.scalar.dma_start(out=wv_sb, in_=wv_d[:])
        ld(nc.scalar, xb_sb, xb_d, 1, 0)
        ld(nc.scalar, xb_sb, xb_d, 1, 1)

        ones_b = singles.tile([P, P], BF16)   # all-ones bf16 lhsT: Z colsum
        nc.vector.memset(ones_b, 1.0)

        # ---- projections --------------------------------------------------------
        # qq/kk stored bf16: the QK matmuls then stream 1 cycle/col with
        # single-pass weight loads.  All projection matmuls are bf16 (fp32r
        # runs 2 cycles/col on HW); q keeps full x precision via the hi+lo
        # split (two accumulating bf16 passes).
        qq_sb = singles.tile([P, NI], BF16)    # [q; q] duplicated across halves
        kk_sb = singles.tile([P, N], BF16)     # [k; k] duplicated across halves
        vt_sb = singles.tile([P, N // P, C], BF16)   # V^T: [j, c], +vb folded

        def qq_slice(s):
            ps = pp.tile([P, 512], F32, tag="pp", name=f"qq_ps_{s}")
            nc.tensor.matmul(ps, wq_sb[:, 0], xb_sb[:, 0, bass.ts(s, 512)],
                             start=True, stop=False)
            nc.tensor.matmul(ps, wq_sb[:, 0], xlo_sb[:, 0, bass.ts(s, 512)],
                             start=False, stop=False)
            nc.tensor.matmul(ps, wq_sb[:, 1], xb_sb[:, 1, bass.ts(s, 512)],
                             start=False, stop=False)
            nc.tensor.matmul(ps, wq_sb[:, 1], xlo_sb[:, 1, bass.ts(s, 512)],
                             start=False, stop=True)
            nc.scalar.activation(out=qq_sb[:, bass.ts(s, 512)], in_=ps,
                                 func=IDENT, bias=qb_sb, scale=1.0)

        def kk_slice(s):
            ps = pp.tile([P, 512], F32, tag="pp", name=f"kk_ps_{s}")
            nc.tensor.matmul(ps, wk_sb[:, 0], xb_sb[:, 0, bass.ts(s, 512)],
                             start=True, stop=False)
            nc.tensor.matmul(ps, wk_sb[:, 1], xb_sb[:, 1, bass.ts(s, 512)],
                             start=False, stop=True)
            nc.scalar.activation(out=kk_sb[:, bass.ts(s, 512)], in_=ps,
                                 func=IDENT, bias=kb_sb, scale=1.0)

        def vt_chunk(j):
            ps = pp.tile([P, C], F32, tag="pp", name=f"vt_ps_{j}")
            nc.tensor.matmul(ps, xb_sb[:, 0, bass.ts(j, P)], wv_sb[:, 0],
                             start=True, stop=False)
            nc.tensor.matmul(ps, xb_sb[:, 1, bass.ts(j, P)], wv_sb[:, 1],
                             start=False, stop=True)
            nc.vector.tensor_add(vt_sb[:, j], ps, vb_bc)

        # consume xb strictly in chunk-arrival order; the o=1 half of
        # chunk s+2 is issued from the scalar queue as chunk s is consumed
        # (chunks 6-7 ride the Pool queue instead); qq slices 1-3 are
        # emitted mid-phase, right before the pipeline needs them
        kk_slice(0)
        for j in range(4):
            vt_chunk(j)
        qq_slice(0)
        for s in range(1, N // 512):
            if s + 1 < 6:
                ld(nc.scalar, xb_sb, xb_d, 1, s + 1)
            kk_slice(s)
            for j in range(4 * s, 4 * s + 4):
                vt_chunk(j)
        qq_slice(1)
        qq_slice(2)
        qq_slice(3)

        # full residual r = xb + xlo (exact fp32 x), precomputed on Pool
        # (idle during projections) so each phase's finalize needs only one
        # add per channel half; chunked to follow the xlo DMA arrivals
        r_sb = singles.tile([P, 2, NI], F32)
        for c in range(4):
            nc.gpsimd.tensor_add(r_sb[:, :, bass.ts(c, 512)],
                                 xb_sb[:, :, bass.ts(c, 512)],
                                 xlo_sb[:, :, bass.ts(c, 512)])

        # ---- attention ----------------------------------------------------------
        # Row-packed QK: pair (jA, jB) = (2t, 2t+1); jA on PE rows 0-63, jB on
        # rows 64-127 (via the duplicated q/k halves), running concurrently.
        NPAIR = N // P // 2   # 16 pairs per phase
        NPH = NI // PH        # 4 phases

        def issue_pair(ph, t):
            # One PSUM tile holds both halves' S^T slices ([P, 2, 512]); the
            # two K=64 QK matmuls row-pack (rows 0-63 / 64-127) and a SINGLE
            # [128, 1024] exp covers both halves (amortizes ScalarE's fixed
            # per-instruction overhead -- ScalarE is the co-critical engine).
            i0 = ph * PH
            ps = p_s.tile([P, 2, PH], F32, tag="s", name=f"ps_{ph}_{t}")
            for h, j in ((0, 2 * t), (1, 2 * t + 1)):
                lo = h * CQ
                nc.tensor.matmul(
                    ps[:, h],
                    kk_sb[lo:lo + CQ, bass.ts(j, P)],
                    qq_sb[lo:lo + CQ, bass.ds(i0, PH)],
                    start=True, stop=True)
            et = etp.tile([P, 2, PH], BF16, tag="et", name=f"et_{ph}_{t}")
            nc.scalar.activation(out=et, in_=ps, func=EXP, scale=1.0)
            return et

        def pv_half(po, t, h, et):
            j = 2 * t + h
            for cc in range(C // P):
                nc.tensor.matmul(
                    po[cc],
                    vt_sb[:, j, bass.ts(cc, P)],
                    et[:, h],
                    start=(t == 0 and h == 0), stop=(t == NPAIR - 1 and h == 1))

        def finalize_z(ph, z2, et15=None):
            # Z colsum + partition-broadcast via all-ones bf16 lhsT matmuls
            # accumulated into one PSUM tile; emitted between the last
            # pair's two PV halves so the reciprocal overlaps the remaining
            # PV stream.  For the final phase the last pair's exp tile is
            # summed directly by the PE (et15) so the colsum can issue a
            # full pair earlier, pulling the reciprocal off the tail.
            pzb = p_s.tile([P, PH], F32, tag="s", name=f"pzb_{ph}")
            nc.tensor.matmul(pzb, ones_b, z2[:, 0], start=True, stop=False)
            nc.tensor.matmul(pzb, ones_b, z2[:, 1],
                             start=False, stop=et15 is None)
            if et15 is not None:
                nc.tensor.matmul(pzb, ones_b, et15[:, 0],
                                 start=False, stop=False)
                nc.tensor.matmul(pzb, ones_b, et15[:, 1],
                                 start=False, stop=True)
            zbc = zp.tile([P, PH], F32, tag="zbc", name=f"zbc_{ph}")
            nc.vector.reciprocal_approx_fast(out=zbc, in_=pzb)
            return zbc

        def finalize_out(ph, po, zbc):
            # ob = (po * gamma) * (1/Z) via one STT per channel half, then a
            # single +r residual add.  Middle phases put the adds on Pool
            # while the PE begins the next phase (po PSUM banks rotate
            # between phases); the last phase keeps them on Vector (shorter
            # tail).  DMA issues ride the sync queue and the engine that
            # produced the data -- NEVER the scalar queue, whose in-order
            # stream would stall the critical exps behind the DMA's wait.
            last = ph == NPH - 1
            sl_i = bass.ds(ph * PH, PH)
            ob = outp.tile([P, 2, PH], F32, tag="ob", name=f"ob_{ph}")
            ob2 = outp.tile([P, 2, PH], BF16, tag="ob2", name=f"ob2_{ph}")
            for cc in range(C // P):
                nc.vector.scalar_tensor_tensor(
                    out=ob[:, cc], in0=po[cc], scalar=gamma_bc, in1=zbc,
                    op0=MUL, op1=MUL)
                # single bf16 rounding at the very end (out ships bf16,
                # halving the output DMA; host upcasts)
                if last:
                    nc.vector.tensor_add(ob2[:, cc], ob[:, cc],
                                         r_sb[:, cc, sl_i])
                else:
                    nc.gpsimd.tensor_add(ob2[:, cc], ob[:, cc],
                                         r_sb[:, cc, sl_i])
                if cc == 0:
                    q = nc.sync
                else:
                    q = nc.scalar if last else nc.gpsimd
                q.dma_start(out=out_d[:][:, cc:cc + 1, sl_i],
                            in_=ob2[:, cc:cc + 1])

        # Software pipeline over double-pair steps: per step, issue QK/exp
        # for the NEXT two pairs back-to-back, then run all 8 PV matmuls of
        # the current two pairs.  Batching the QK groups halves the number of
        # QK<->PV transitions (each costs ~300-400ns of weight-load/drain
        # exposure), and the ~2.2us of exp still fits inside the step.
        all_pairs = [(ph, t) for ph in range(NPH) for t in range(NPAIR)]
        pend = {pr: issue_pair(*pr) for pr in all_pairs[:2]}
        z2_by_ph = {}
        po_by_ph = {}
        for i in range(0, len(all_pairs), 2):
            pra, prb = all_pairs[i], all_pairs[i + 1]
            ph = pra[0]
            if pra[1] == 0:
                z2_by_ph[ph] = zp.tile([P, 2, PH], BF16, tag="z2",
                                       name=f"z2_{ph}")
                po_by_ph[ph] = [pp.tile([P, PH], F32, tag="pp",
                                        name=f"po_{ph}_{cc}")
                                for cc in range(C // P)]
            z2 = z2_by_ph[ph]
            po = po_by_ph[ph]
            eta, etb = pend.pop(pra), pend.pop(prb)
            for pr in all_pairs[i + 2:i + 4]:
                pend[pr] = issue_pair(*pr)
            last_ph = ph == NPH - 1
            for t, et in ((pra[1], eta), (prb[1], etb)):
                if t == 0:
                    nc.vector.tensor_copy(z2, et)
                elif not (last_ph and t == NPAIR - 1):
                    nc.vector.tensor_add(z2, z2, et)
                if t == NPAIR - 1:
                    if last_ph:
                        # 4-term colsum before pair 15's PVs: the reciprocal
                        # and the finalize chain leave the kernel tail
                        zbc = finalize_z(ph, z2, et15=et)
                        pv_half(po, t, 0, et)
                        pv_half(po, t, 1, et)
                    else:
                        pv_half(po, t, 0, et)
                        zbc = finalize_z(ph, z2)
                        pv_half(po, t, 1, et)
                    finalize_out(ph, po, zbc)
                else:
                    pv_half(po, t, 0, et)
                    pv_half(po, t, 1, et)


def _build_nc():
    nc = bacc.Bacc(trn_type="TRN2", target_bir_lowering=False, debug=False)
    aps = (
        nc.declare_dram_parameter("xb", [P, 2, N], BF16, isOutput=False),
        nc.declare_dram_parameter("xlo", [P, 2, NI], BF16, isOutput=False),
        nc.declare_dram_parameter("wq", [P, 2, P], BF16, isOutput=False),
        nc.declare_dram_parameter("wk", [P, 2, P], BF16, isOutput=False),
        nc.declare_dram_parameter("wv", [P, 2, C], BF16, isOutput=False),
        nc.declare_dram_parameter("qb", [P, 1], F32, isOutput=False),
        nc.declare_dram_parameter("kb", [P, 1], F32, isOutput=False),
        nc.declare_dram_parameter("vb_bc", [P, C], F32, isOutput=False),
        nc.declare_dram_parameter("gamma_bc", [P, 1], F32, isOutput=False),
        nc.declare_dram_parameter("out", [P, 2, NI], BF16, isOutput=True),
    )
    with tile.TileContext(nc) as tc:
        _emit(tc, aps)
    nc.compile()
    return nc


_NC_CACHE = {}


def get_nc():
    if "nc" not in _NC_CACHE:
        _NC_CACHE["nc"] = _build_nc()
    return _NC_CACHE["nc"]


def _pmajor(a, free):
    """[256, free] -> contiguous [128, 2, free] partition-major view."""
    return np.ascontiguousarray(a.reshape(2, P, free).transpose(1, 0, 2))


def make_in_maps(inputs):
    """Build the 8 per-core input maps from the full problem inputs."""
    import ml_dtypes
    f = np.float32
    bf = ml_dtypes.bfloat16
    x_streams = [
        np.ascontiguousarray(inputs["input1"].reshape(B, C, N), dtype=f),
        np.ascontiguousarray(inputs["input2"].reshape(B, C, N), dtype=f),
    ]
    wsets = []
    for s in ("1", "2"):
        qw = np.asarray(inputs[f"q{s}_w"], dtype=f)
        kw = np.asarray(inputs[f"k{s}_w"], dtype=f)
        vw = np.asarray(inputs[f"v{s}_w"], dtype=f)
        qb = np.asarray(inputs[f"q{s}_b"], dtype=f)
        kb = np.asarray(inputs[f"k{s}_b"], dtype=f)
        vb = np.asarray(inputs[f"v{s}_b"], dtype=f)
        wsets.append(dict(
            wq=_pmajor(np.concatenate([qw, qw], 0).T.astype(bf), P),
            wk=_pmajor(np.concatenate([kw, kw], 0).T.astype(bf), P),
            wv=_pmajor(vw.T.astype(bf), C),
            qb=np.ascontiguousarray(np.concatenate([qb, qb])[:, None]),
            kb=np.ascontiguousarray(np.concatenate([kb, kb])[:, None]),
            vb_bc=np.ascontiguousarray(np.broadcast_to(vb[None, :], (P, C)),
                                       dtype=f),
        ))
    gamma = np.asarray(inputs["gamma"], dtype=f).reshape(1, 1)
    gamma_bc = np.ascontiguousarray(np.broadcast_to(gamma, (P, 1)))

    in_maps = []
    for core in range(8):
        u, h = core // 2, core % 2
        b, s = u // 2, u % 2
        xs = x_streams[s][b]
        # rotate so this core's 2048 query columns come first (attention
        # contracts over all keys, so key/value column order is irrelevant)
        xrot = np.concatenate([xs[:, h * NI:], xs[:, :h * NI]], axis=1)
        xb = xrot.astype(bf)
        xlo = (xrot[:, :NI] - xb[:, :NI].astype(f)).astype(bf)
        m = dict(wsets[s])
        m["xb"] = _pmajor(xb, N)
        m["xlo"] = _pmajor(xlo, NI)
        m["gamma_bc"] = gamma_bc
        in_maps.append(m)
    return in_maps


def assemble(results, inputs):
    """Stitch the 8 per-core [128, 2, 2048] outputs into (out1, out2)."""
    outs = [np.empty((B, C, N), np.float32) for _ in range(2)]
    for core in range(8):
        u, h = core // 2, core % 2
        b, s = u // 2, u % 2
        o = results[core]["out"].astype(np.float32).transpose(1, 0, 2).reshape(C, NI)
        outs[s][b][:, h * NI:(h + 1) * NI] = o
    out1 = outs[0].reshape(B, C, H, W)
    out2 = outs[1].reshape(B, C, H, W)
    return out1, out2


def kernel(**inputs):
    from concourse.bass_utils import run_bass_kernel_spmd

    nc = get_nc()
    in_maps = make_in_maps(inputs)
    res = run_bass_kernel_spmd(nc, in_maps, list(range(8)))
    return assemble(res.results, inputs)


# revision 32
# speedup vs baseline: 1.1182x; 1.0034x over previous
"""Trainium2 Bass kernel for nn_AttShare: dual-stream 1x1-conv attention.

Full-input contract: kernel(**inputs) takes the complete tensors from
setup_inputs() and returns (out1, out2) exactly like the reference.

Sharding (8 cores): 4 independent (batch, stream) attention units x 2-way
query-row split.  Each core gets the full x=[256,4096] of its unit, HOST-
ROTATED so its 2048 query columns come first; it produces
out = gamma * (V @ softmax(Q K^T)^T)[:, 0:2048] + x[:, 0:2048].
(Attention contracts over all keys, so the key/value column order is
irrelevant; the host scatters the output back to the right columns.)

Key simplification: the reference adds a per-row bias (q . g) to the logits
before a row-softmax.  softmax is shift-invariant per row, so the entire
global-gating branch (pooled means -> MLP -> sigmoid -> bias) cancels and is
not computed.  The k-projection bias also shifts logits uniformly per row
and cancels; the q bias does not and is applied.  The v bias is folded
into the V^T tiles; gamma is folded into the finalize
scalar_tensor_tensor (exact); gamma and vb ship host-pre-broadcast so the
kernel needs no cold-start K=1 broadcast matmuls.

Precision: everything rides bf16 except the PSUM accumulations (f32) and
the residual, which is reconstructed exactly from a bf16 hi+lo split of x
(xb + xlo == fp32 x to ~2^-17).  The q projection also uses the hi+lo
split (two bf16 matmul passes == fp32r's measured 2 cycles/col, but no
fp32 DMA); k/v project from xb alone.  The output ships bf16 (single
final rounding) and the host upcasts.  Measured accuracy 1.267e-2
relative (tolerance 2e-2); the dominant term is bf16 rounding of q/k.

On-core dataflow (per core):
  proj:  kk = Wk_dup @ xb (+kb)            [128, 4096] -> bf16 (k dup'd on
         qq = Wq_dup @ (xb+xlo) (+qb)      [128, 2048] -> bf16  both halves
         vt = xb^T Wv^T (+vb via Vector)   [128 j, 32, 256] bf16 for packing)
       (Tile's dataflow scheduler automatically defers late-chunk proj
        work into attention phase 0 as the DMA arrives.)
  attn (4 phases of 512 query columns, keys streamed in row-packed pairs
        of 128-key chunks, software-pipelined in double-pair steps):
         S^T = kk_j^T @ qq  (K=64, PE rows 0-63 / 64-127 run CONCURRENTLY
               via row tiling -- a pair costs ~213ns, not 426; both pairs'
               QK groups issued back-to-back to halve the QK<->PV
               weight-load transitions)
         E = exp(S^T)  (one [128,1024] ScalarE pass per pair covering both
               halves; ScalarE is co-critical with the PE at ~18us/phase;
               no max-shift needed: |S|<~60 in bf16 and the denominator
               normalizes later)
         z2 += E  (Vector, one [128,1024] bf16 add per pair)
         out_psum[c,i] += vt_j^T @ E  (bf16 matmuls, PSUM-resident)
  finalize per phase: Z colsum+broadcast via all-ones bf16 lhsT matmuls
  (middle phases: 2 matmuls between the last pair's PV halves; final
  phase: 4 matmuls incl. the last exp tile directly, emitted BEFORE the
  last PVs so the reciprocal+STT chain leaves the kernel tail), fast
  reciprocal (Vector, 18-bit), ob = (po*gamma)*recip via STT, +r residual
  (r = xb+xlo precomputed on Pool), single bf16 rounding, DMA out.
  Middle phases put the +r adds on Pool while the PE starts the next
  phase; DMA issues ride sync/pool queues (and scalar only at the very
  end, once no exps remain) -- a DMA wait on the scalar queue mid-kernel
  would stall the critical exps behind it.
  PSUM budget 8 banks: 2x2 phase-rotated output accumulators + 2x2-bank
  S^T slots shared with the projection psums and Z colsums.

Head: all tensors are pre-permuted on host to partition-major [128, 2, n]
bf16 so every DMA is a dense descriptor.  Input bytes are spread over all
three DMA queues (~1.1MB each at ~130GB/s): SP carries x o=0 halves
(issue-only engine, front-loaded), Activation carries o=1 halves with the
descriptor writes paced inside the projection emission, Pool carries the
tiny params, pre-broadcasts, xlo, and the last two x chunks.
"""

import os
import sys

import numpy as np

for _p in ("/opt/trn_rl_repo", os.path.expanduser("~/.axon_site/_ro/trn_rl_repo")):
    if os.path.isdir(_p) and _p not in sys.path:
        sys.path.insert(0, _p)

import concourse.bass as bass  # noqa: E402
import concourse.bacc as bacc  # noqa: E402
import concourse.mybir as mybir  # noqa: E402
import concourse.tile as tile  # noqa: E402

P = 128
C = 256         # channels
CQ = 64         # q/k channels
N = 4096        # H*W
NI = 2048       # query rows per core
PH = 512        # query columns processed per phase
B, H, W = 2, 64, 64
F32 = mybir.dt.float32
BF16 = mybir.dt.bfloat16


def _emit(tc, aps):
    nc = tc.nc
    import contextlib

    (xb_d, xlo_d, wq_d, wk_d, wv_d, qb_d, kb_d, vbb_d, gbc_d, out_d) = aps
    EXP = mybir.ActivationFunctionType.Exp
    IDENT = mybir.ActivationFunctionType.Identity
    MUL = mybir.AluOpType.mult
    ADD = mybir.AluOpType.add

    with contextlib.ExitStack() as ctx:
        singles = ctx.enter_context(tc.tile_pool(name="singles", bufs=1))
        pp = ctx.enter_context(tc.tile_pool(name="pp", bufs=4, space="PSUM"))
        p_s = ctx.enter_context(tc.tile_pool(name="p_s", bufs=2, space="PSUM"))
        etp = ctx.enter_context(tc.tile_pool(name="etp", bufs=8))
        zp = ctx.enter_context(tc.tile_pool(name="zp", bufs=4))
        outp = ctx.enter_context(tc.tile_pool(name="outp", bufs=4))

        # ---- loads --------------------------------------------------------------
        xb_sb = singles.tile([P, 2, N], BF16)     # full rotated x, bf16 hi
        xlo_sb = singles.tile([P, 2, NI], BF16)   # query-half lo correction
        wq_sb = singles.tile([P, 2, P], BF16)
        wk_sb = singles.tile([P, 2, P], BF16)
        wv_sb = singles.tile([P, 2, C], BF16)

        kb_sb = singles.tile([P, 1], F32)
        qb_sb = singles.tile([P, 1], F32)
        gamma_bc = singles.tile([P, 1], F32)   # host pre-broadcast
        vb_bc = singles.tile([P, C], F32)      # host pre-broadcast

        def ld(queue, sb, dram, o, c):
            queue.dma_start(out=sb[:, o:o + 1, bass.ts(c, 512)],
                            in_=dram[:][:, o:o + 1, bass.ts(c, 512)])

        def ld2(queue, sb, dram, c):
            queue.dma_start(out=sb[:, :, bass.ts(c, 512)],
                            in_=dram[:][:, :, bass.ts(c, 512)])

        # Three-queue head: SP carries o=0 x halves (issue-only engine,
        # front-loaded); Activation carries o=1 halves, paced inside the
        # projection emission so descriptor writes never starve the
        # identity activations; Pool carries the tiny params, both
        # pre-broadcasts, xlo, and the last two x chunks.  ~1.1MB/queue.
        nc.gpsimd.dma_start(out=qb_sb, in_=qb_d[:])
        nc.gpsimd.dma_start(out=kb_sb, in_=kb_d[:])
        nc.gpsimd.dma_start(out=gamma_bc, in_=gbc_d[:])
        nc.gpsimd.dma_start(out=vb_bc, in_=vbb_d[:])
        nc.gpsimd.dma_start(out=xlo_sb[:, :, bass.ds(0, 512)],
                            in_=xlo_d[:][:, :, bass.ds(0, 512)])
        ld2(nc.gpsimd, xb_sb, xb_d, 6)
        ld2(nc.gpsimd, xb_sb, xb_d, 7)
        nc.gpsimd.dma_start(out=xlo_sb[:, :, bass.ds(512, 1536)],
                            in_=xlo_d[:][:, :, bass.ds(512, 1536)])

        nc.sync.dma_start(out=wk_sb, in_=wk_d[:])
        nc.sync.dma_start(out=wq_sb, in_=wq_d[:])
        for c in range(6):
            ld(nc.sync, xb_sb, xb_d, 0, c)
        nc.scalar.dma_start(out=wv_sb, in_=wv_d[:])
        ld(nc.scalar, xb_sb, xb_d, 1, 0)
        ld(nc.scalar, xb_sb, xb_d, 1, 1)

        ones_b = singles.tile([P, P], BF16)   # all-ones bf16 lhsT: Z colsum
        nc.vector.memset(ones_b, 1.0)

        # ---- projections --------------------------------------------------------
        # qq/kk stored bf16: the QK matmuls then stream 1 cycle/col with
        # single-pass weight loads.  All projection matmuls are bf16 (fp32r
        # runs 2 cycles/col on HW); q keeps full x precision via the hi+lo
        # split (two accumulating bf16 passes).
        qq_sb = singles.tile([P, NI], BF16)    # [q; q] duplicated across halves
        kk_sb = singles.tile([P, N], BF16)     # [k; k] duplicated across halves
        vt_sb = singles.tile([P, N // P, C], BF16)   # V^T: [j, c], +vb folded

        def qq_slice(s):
            ps = pp.tile([P, 512], F32, tag="pp", name=f"qq_ps_{s}")
            nc.tensor.matmul(ps, wq_sb[:, 0], xb_sb[:, 0, bass.ts(s, 512)],
                             start=True, stop=False)
            nc.tensor.matmul(ps, wq_sb[:, 0], xlo_sb[:, 0, bass.ts(s, 512)],
                             start=False, stop=False)
            nc.tensor.matmul(ps, wq_sb[:, 1], xb_sb[:, 1, bass.ts(s, 512)],
                             start=False, stop=False)
            nc.tensor.matmul(ps, wq_sb[:, 1], xlo_sb[:, 1, bass.ts(s, 512)],
                             start=False, stop=True)
            nc.vector.tensor_scalar_add(qq_sb[:, bass.ts(s, 512)], ps, qb_sb)

        def kk_slice(s):
            ps = pp.tile([P, 512], F32, tag="pp", name=f"kk_ps_{s}")
            nc.tensor.matmul(ps, wk_sb[:, 0], xb_sb[:, 0, bass.ts(s, 512)],
                             start=True, stop=False)
            nc.tensor.matmul(ps, wk_sb[:, 1], xb_sb[:, 1, bass.ts(s, 512)],
                             start=False, stop=True)
            nc.vector.tensor_scalar_add(kk_sb[:, bass.ts(s, 512)], ps, kb_sb)

        def vt_chunk(j):
            ps = pp.tile([P, C], F32, tag="pp", name=f"vt_ps_{j}")
            nc.tensor.matmul(ps, xb_sb[:, 0, bass.ts(j, P)], wv_sb[:, 0],
                             start=True, stop=False)
            nc.tensor.matmul(ps, xb_sb[:, 1, bass.ts(j, P)], wv_sb[:, 1],
                             start=False, stop=True)
            nc.vector.tensor_add(vt_sb[:, j], ps, vb_bc)

        # consume xb strictly in chunk-arrival order; the o=1 half of
        # chunk s+2 is issued from the scalar queue as chunk s is consumed
        # (chunks 6-7 ride the Pool queue instead); qq slices 1-3 are
        # emitted mid-phase, right before the pipeline needs them
        kk_slice(0)
        for j in range(4):
            vt_chunk(j)
        qq_slice(0)
        for s in range(1, N // 512):
            if s + 1 < 6:
                ld(nc.scalar, xb_sb, xb_d, 1, s + 1)
            kk_slice(s)
            for j in range(4 * s, 4 * s + 4):
                vt_chunk(j)
        qq_slice(1)
        qq_slice(2)
        qq_slice(3)

        # full residual r = xb + xlo (exact fp32 x), precomputed on Pool
        # (idle during projections) so each phase's finalize needs only one
        # add per channel half; chunked to follow the xlo DMA arrivals
        r_sb = singles.tile([P, 2, NI], F32)
        for c in range(4):
            nc.gpsimd.tensor_add(r_sb[:, :, bass.ts(c, 512)],
                                 xb_sb[:, :, bass.ts(c, 512)],
                                 xlo_sb[:, :, bass.ts(c, 512)])

        # ---- attention ----------------------------------------------------------
        # Row-packed QK: pair (jA, jB) = (2t, 2t+1); jA on PE rows 0-63, jB on
        # rows 64-127 (via the duplicated q/k halves), running concurrently.
        NPAIR = N // P // 2   # 16 pairs per phase
        NPH = NI // PH        # 4 phases

        def issue_pair(ph, t):
            # One PSUM tile holds both halves' S^T slices ([P, 2, 512]); the
            # two K=64 QK matmuls row-pack (rows 0-63 / 64-127) and a SINGLE
            # [128, 1024] exp covers both halves (amortizes ScalarE's fixed
            # per-instruction overhead -- ScalarE is the co-critical engine).
            i0 = ph * PH
            ps = p_s.tile([P, 2, PH], F32, tag="s", name=f"ps_{ph}_{t}")
            for h, j in ((0, 2 * t), (1, 2 * t + 1)):
                lo = h * CQ
                nc.tensor.matmul(
                    ps[:, h],
                    kk_sb[lo:lo + CQ, bass.ts(j, P)],
                    qq_sb[lo:lo + CQ, bass.ds(i0, PH)],
                    start=True, stop=True)
            et = etp.tile([P, 2, PH], BF16, tag="et", name=f"et_{ph}_{t}")
            nc.scalar.activation(out=et, in_=ps, func=EXP, scale=1.0)
            return et

        def pv_half(po, t, h, et):
            j = 2 * t + h
            for cc in range(C // P):
                nc.tensor.matmul(
                    po[cc],
                    vt_sb[:, j, bass.ts(cc, P)],
                    et[:, h],
                    start=(t == 0 and h == 0), stop=(t == NPAIR - 1 and h == 1))

        def finalize_z(ph, z2, et15=None):
            # Z colsum + partition-broadcast via all-ones bf16 lhsT matmuls
            # accumulated into one PSUM tile; emitted between the last
            # pair's two PV halves so the reciprocal overlaps the remaining
            # PV stream.  For the final phase the last pair's exp tile is
            # summed directly by the PE (et15) so the colsum can issue a
            # full pair earlier, pulling the reciprocal off the tail.
            pzb = p_s.tile([P, PH], F32, tag="s", name=f"pzb_{ph}")
            nc.tensor.matmul(pzb, ones_b, z2[:, 0], start=True, stop=False)
            nc.tensor.matmul(pzb, ones_b, z2[:, 1],
                             start=False, stop=et15 is None)
            if et15 is not None:
                nc.tensor.matmul(pzb, ones_b, et15[:, 0],
                                 start=False, stop=False)
                nc.tensor.matmul(pzb, ones_b, et15[:, 1],
                                 start=False, stop=True)
            zbc = zp.tile([P, PH], F32, tag="zbc", name=f"zbc_{ph}")
            nc.vector.reciprocal_approx_fast(out=zbc, in_=pzb)
            return zbc

        def finalize_out(ph, po, zbc):
            # ob = (po * gamma) * (1/Z) via one STT per channel half, then a
            # single +r residual add.  Middle phases put the adds on Pool
            # while the PE begins the next phase (po PSUM banks rotate
            # between phases); the last phase keeps them on Vector (shorter
            # tail).  DMA issues ride the sync queue and the engine that
            # produced the data -- NEVER the scalar queue, whose in-order
            # stream would stall the critical exps behind the DMA's wait.
            last = ph == NPH - 1
            sl_i = bass.ds(ph * PH, PH)
            ob = outp.tile([P, 2, PH], F32, tag="ob", name=f"ob_{ph}")
            ob2 = outp.tile([P, 2, PH], BF16, tag="ob2", name=f"ob2_{ph}")
            for cc in range(C // P):
                nc.vector.scalar_tensor_tensor(
                    out=ob[:, cc], in0=po[cc], scalar=gamma_bc, in1=zbc,
                    op0=MUL, op1=MUL)
                # single bf16 rounding at the very end (out ships bf16,
                # halving the output DMA; host upcasts)
                if last:
                    nc.vector.tensor_add(ob2[:, cc], ob[:, cc],
                                         r_sb[:, cc, sl_i])
                else:
                    nc.gpsimd.tensor_add(ob2[:, cc], ob[:, cc],
                                         r_sb[:, cc, sl_i])
                if cc == 0:
                    q = nc.sync
                else:
                    q = nc.scalar if last else nc.gpsimd
                q.dma_start(out=out_d[:][:, cc:cc + 1, sl_i],
                            in_=ob2[:, cc:cc + 1])

        # Software pipeline over double-pair steps: per step, issue QK/exp
        # for the NEXT two pairs back-to-back, then run all 8 PV matmuls of
        # the current two pairs.  Batching the QK groups halves the number of
        # QK<->PV transitions (each costs ~300-400ns of weight-load/drain
        # exposure), and the ~2.2us of exp still fits inside the step.
        all_pairs = [(ph, t) for ph in range(NPH) for t in range(NPAIR)]
        pend = {pr: issue_pair(*pr) for pr in all_pairs[:2]}
        z2_by_ph = {}
        po_by_ph = {}
        for i in range(0, len(all_pairs), 2):
            pra, prb = all_pairs[i], all_pairs[i + 1]
            ph = pra[0]
            if pra[1] == 0:
                z2_by_ph[ph] = zp.tile([P, 2, PH], BF16, tag="z2",
                                       name=f"z2_{ph}")
                po_by_ph[ph] = [pp.tile([P, PH], F32, tag="pp",
                                        name=f"po_{ph}_{cc}")
                                for cc in range(C // P)]
            z2 = z2_by_ph[ph]
            po = po_by_ph[ph]
            eta, etb = pend.pop(pra), pend.pop(prb)
            for pr in all_pairs[i + 2:i + 4]:
                pend[pr] = issue_pair(*pr)
            last_ph = ph == NPH - 1
            for t, et in ((pra[1], eta), (prb[1], etb)):
                if t == 0:
                    nc.vector.tensor_copy(z2, et)
                elif not (last_ph and t == NPAIR - 1):
                    nc.vector.tensor_add(z2, z2, et)
                if t == NPAIR - 1:
                    if last_ph:
                        # 4-term colsum before pair 15's PVs: the reciprocal
                        # and the finalize chain leave the kernel tail
                        zbc = finalize_z(ph, z2, et15=et)
                        pv_half(po, t, 0, et)
                        pv_half(po, t, 1, et)
                    else:
                        pv_half(po, t, 0, et)
                        zbc = finalize_z(ph, z2)
                        pv_half(po, t, 1, et)
                    finalize_out(ph, po, zbc)
                else:
                    pv_half(po, t, 0, et)
                    pv_half(po, t, 1, et)


def _build_nc():
    nc = bacc.Bacc(trn_type="TRN2", target_bir_lowering=False, debug=False)
    aps = (
        nc.declare_dram_parameter("xb", [P, 2, N], BF16, isOutput=False),
        nc.declare_dram_parameter("xlo", [P, 2, NI], BF16, isOutput=False),
        nc.declare_dram_parameter("wq", [P, 2, P], BF16, isOutput=False),
        nc.declare_dram_parameter("wk", [P, 2, P], BF16, isOutput=False),
        nc.declare_dram_parameter("wv", [P, 2, C], BF16, isOutput=False),
        nc.declare_dram_parameter("qb", [P, 1], F32, isOutput=False),
        nc.declare_dram_parameter("kb", [P, 1], F32, isOutput=False),
        nc.declare_dram_parameter("vb_bc", [P, C], F32, isOutput=False),
        nc.declare_dram_parameter("gamma_bc", [P, 1], F32, isOutput=False),
        nc.declare_dram_parameter("out", [P, 2, NI], BF16, isOutput=True),
    )
    with tile.TileContext(nc) as tc:
        _emit(tc, aps)
    nc.compile()
    return nc


_NC_CACHE = {}


def get_nc():
    if "nc" not in _NC_CACHE:
        _NC_CACHE["nc"] = _build_nc()
    return _NC_CACHE["nc"]


def _pmajor(a, free):
    """[256, free] -> contiguous [128, 2, free] partition-major view."""
    return np.ascontiguousarray(a.reshape(2, P, free).transpose(1, 0, 2))


def make_in_maps(inputs):
    """Build the 8 per-core input maps from the full problem inputs."""
    import ml_dtypes
    f = np.float32
    bf = ml_dtypes.bfloat16
    x_streams = [
        np.ascontiguousarray(inputs["input1"].reshape(B, C, N), dtype=f),
        np.ascontiguousarray(inputs["input2"].reshape(B, C, N), dtype=f),
    ]
    wsets = []
    for s in ("1", "2"):
        qw = np.asarray(inputs[f"q{s}_w"], dtype=f)
        kw = np.asarray(inputs[f"k{s}_w"], dtype=f)
        vw = np.asarray(inputs[f"v{s}_w"], dtype=f)
        qb = np.asarray(inputs[f"q{s}_b"], dtype=f)
        kb = np.asarray(inputs[f"k{s}_b"], dtype=f)
        vb = np.asarray(inputs[f"v{s}_b"], dtype=f)
        wsets.append(dict(
            wq=_pmajor(np.concatenate([qw, qw], 0).T.astype(bf), P),
            wk=_pmajor(np.concatenate([kw, kw], 0).T.astype(bf), P),
            wv=_pmajor(vw.T.astype(bf), C),
            qb=np.ascontiguousarray(np.concatenate([qb, qb])[:, None]),
            kb=np.ascontiguousarray(np.concatenate([kb, kb])[:, None]),
            vb_bc=np.ascontiguousarray(np.broadcast_to(vb[None, :], (P, C)),
                                       dtype=f),
        ))
    gamma = np.asarray(inputs["gamma"], dtype=f).reshape(1, 1)
    gamma_bc = np.ascontiguousarray(np.broadcast_to(gamma, (P, 1)))

    in_maps = []
    for core in range(8):
        u, h = core // 2, core % 2
        b, s = u // 2, u % 2
        xs = x_streams[s][b]
        # rotate so this core's 2048 query columns come first (attention
        # contracts over all keys, so key/value column order is irrelevant)
        xrot = np.concatenate([xs[:, h * NI:], xs[:, :h * NI]], axis=1)
        xb = xrot.astype(bf)
        xlo = (xrot[:, :NI] - xb[:, :NI].astype(f)).astype(bf)
        m = dict(wsets[s])
        m["xb"] = _pmajor(xb, N)
        m["xlo"] = _pmajor(xlo, NI)
        m["gamma_bc"] = gamma_bc
        in_maps.append(m)
    return in_maps


def assemble(results, inputs):
    """Stitch the 8 per-core [128, 2, 2048] outputs into (out1, out2)."""
    outs = [np.empty((B, C, N), np.float32) for _ in range(2)]
    for core in range(8):
        u, h = core // 2, core % 2
        b, s = u // 2, u % 2
        o = results[core]["out"].astype(np.float32).transpose(1, 0, 2).reshape(C, NI)
        outs[s][b][:, h * NI:(h + 1) * NI] = o
    out1 = outs[0].reshape(B, C, H, W)
    out2 = outs[1].reshape(B, C, H, W)
    return out1, out2


def kernel(**inputs):
    from concourse.bass_utils import run_bass_kernel_spmd

    nc = get_nc()
    in_maps = make_in_maps(inputs)
    res = run_bass_kernel_spmd(nc, in_maps, list(range(8)))
    return assemble(res.results, inputs)
